# revision 15
# baseline (speedup 1.0000x reference)
"""Rotary multi-head attention (b=8, n=1024, dim=768, heads=12, d_head=64)
on 8 Trainium2 NeuronCores, data-parallel over batch (1 batch row per core).

v6: PSUM re-plan to decouple the score pipeline from qk production.
  - Attention runs per (pair, query-half): every PSUM tile is ONE bank.
    stp ring: 4x [128,512] (tag stp) -> scores run 2 steps ahead of exp.
    qk acc / vp / pt / op: own 2-slot tag "acc" -> DVE rotary muls no
    longer gate the score matmuls (v5's 3us LDW stalls).
    av: 2 live banks (per-nh lifetime), 2-slot ring.
  - Scalar runs only exp during attention ([128,512] granularity).
  - Rotated q/k kept f32 through the swap; single end-stage bf16 round
    into qf, which the bf16 score matmuls read (FWL weight loads).
  - Normalize: batched [128,16] reciprocal, DRAM bounce, one [64,2048]
    broadcast read, divide on GpSimd (keeps Vector FIFO clear).
"""
import sys
import numpy as np

if '/opt/trn_rl_repo' not in sys.path:
    sys.path.insert(0, '/opt/trn_rl_repo')

B, N, DIM = 8, 1024, 768
HEADS, DHEAD = 12, 64
INNER = HEADS * DHEAD           # 768
SCALE = DHEAD ** -0.5           # 0.125
NCH = N // 128                  # 8 n-chunks
KCH = DIM // 128                # 6 contraction chunks
TCH = HEADS // 2                # 6 head pairs

_CACHE = {}


def _build():
    import concourse.mybir as mybir
    from concourse import bacc
    from concourse.tile import TileContext

    F32 = mybir.dt.float32
    F32R = mybir.dt.float32r
    BF16 = mybir.dt.bfloat16
    AF = mybir.ActivationFunctionType

    nc = bacc.Bacc("TRN2", target_bir_lowering=False, debug=False, num_devices=8)

    x_d = nc.dram_tensor("x", [N, DIM], F32, kind="ExternalInput")
    pos_d = nc.dram_tensor("pos", [N, DHEAD], F32, kind="ExternalInput")
    wqkv_d = nc.dram_tensor("wqkv", [DIM, 3 * INNER], F32R, kind="ExternalInput")
    wout_d = nc.dram_tensor("wout", [INNER, DIM], F32, kind="ExternalInput")
    bout_d = nc.dram_tensor("bout", [DIM], F32, kind="ExternalInput")
    y_d = nc.dram_tensor("y", [N, DIM], F32, kind="ExternalOutput")
    den_d = nc.dram_tensor("den_scr", [TCH, 4, 128, 4], F32)

    # ---- inline constants -------------------------------------------------
    ident_d = nc.inline_tensor(np.eye(128, dtype=np.float32), name="ident")
    # sin table with pair-swap sign baked in: odd rows get -sin.
    # posT rows 0:32 = sin values, 32:64 = cos values.
    RsinT = np.zeros((64, 128), np.float32)
    RcosT = np.zeros((64, 128), np.float32)
    for m in range(128):
        RsinT[(m % 64) // 2, m] = -1.0 if (m % 2 == 1) else 1.0
        RcosT[32 + (m % 64) // 2, m] = 1.0
    rsinT_d = nc.inline_tensor(RsinT, name="rsinT")
    rcosT_d = nc.inline_tensor(RcosT, name="rcosT")

    with TileContext(nc) as tc:
        with tc.tile_pool(name="wp", bufs=1) as wp, \
             tc.tile_pool(name="big", bufs=1) as big, \
             tc.tile_pool(name="tp", bufs=2) as tp, \
             tc.tile_pool(name="qpool", bufs=2) as qpool, \
             tc.tile_pool(name="epool", bufs=1) as epool, \
             tc.tile_pool(name="npool", bufs=2) as npool, \
             tc.tile_pool(name="misc", bufs=1) as misc, \
             tc.tile_pool(name="ps", bufs=2, space="PSUM") as ps, \
             tc.tile_pool(name="psv", bufs=2, space="PSUM") as psv:

            # ---- input DMAs.  sync queue: x, consts, Q+Klo weight half.
            # gpsimd queue: pos, rotary tables, Khi+V weight half, wout.
            ident_sb = misc.tile([128, 128], F32, name="ident_sb", tag="ident_sb")
            nc.sync.dma_start(ident_sb[:], ident_d.ap())
            b_row = misc.tile([1, DIM], F32, name="b_row", tag="b_row")
            nc.sync.dma_start(b_row[:], bout_d.ap().unsqueeze(0))
            x_sb = []
            for i in range(NCH):
                xs = tp.tile([128, DIM], F32, name=f"x_sb_{i}", tag="xsb", bufs=3)
                nc.sync.dma_start(xs[:], x_d[i * 128:(i + 1) * 128, :])
                x_sb.append(xs)

            p_all = misc.tile([128, NCH * DHEAD], F32, name="p_all", tag="p_all")
            for i in range(NCH):
                nc.gpsimd.dma_start(p_all[:, i * DHEAD:(i + 1) * DHEAD],
                                    pos_d[i * 128:(i + 1) * 128, :])

            b_bcast = misc.tile([128, DIM], F32, name="b_bcast", tag="b_bcast")
            nc.gpsimd.partition_broadcast(b_bcast[:], b_row[:])

            rs_sb = misc.tile([64, 128], F32R, name="rs_sb", tag="rs_sb")
            nc.gpsimd.dma_start(rs_sb[:], rsinT_d.ap())
            rc_sb = misc.tile([64, 128], F32R, name="rc_sb", tag="rc_sb")
            nc.gpsimd.dma_start(rc_sb[:], rcosT_d.ap())

            wqkv_r = [wp.tile([128, 3 * INNER], F32R, name=f"wqkv_r_{k}",
                              tag=f"wqkv_r_{k}") for k in range(KCH)]
            half_cols = 3 * INNER // 2      # 1152
            for k in range(KCH):
                nc.sync.dma_start(wqkv_r[k][:, 0:half_cols],
                                  wqkv_d[k * 128:(k + 1) * 128, 0:half_cols])
                nc.gpsimd.dma_start(wqkv_r[k][:, half_cols:],
                                    wqkv_d[k * 128:(k + 1) * 128, half_cols:])
            # wout cast to bf16 by the software DGE
            wout_b = [wp.tile([128, DIM], BF16, name=f"wout_b_{k}",
                              tag=f"wout_b_{k}") for k in range(KCH)]
            for k in range(KCH):
                nc.gpsimd.dma_start(wout_b[k][:], wout_d[k * 128:(k + 1) * 128, :])

            # ---- vaug memset (ones columns give softmax denominators)
            vaug_all = big.tile([128, NCH * HEADS * 65], BF16, name="vaug",
                                tag="vaug")
            vaug = [vaug_all[:, i * HEADS * 65:(i + 1) * HEADS * 65]
                    for i in range(NCH)]
            nc.gpsimd.memset(vaug_all[:], 1.0)

            # preload the gpsimd tensor-tensor library off the critical path
            dummy = misc.tile([1, 16], F32, name="dummy_tt", tag="dummy_tt")
            nc.gpsimd.memset(dummy[:], 1.0)
            nc.gpsimd.tensor_add(dummy[:], dummy[:], dummy[:])

            # ---- X^T via PE transposes; PSUM->SBUF copies alternate
            # Scalar/Vector so neither serializes the transpose stream
            xt = [big.tile([128, N], F32R, name=f"xt{k}", tag=f"xt{k}")
                  for k in range(KCH)]

            def emit_transpose_chunk(i):
                pta = ps.tile([128, 512], F32, name=f"pta_{i}", tag="acc")
                ptb = ps.tile([128, 256], F32, name=f"ptb_{i}", tag="acc")
                for k in range(KCH):
                    dst = pta[:, k * 128:(k + 1) * 128] if k < 4 \
                        else ptb[:, (k - 4) * 128:(k - 3) * 128]
                    nc.tensor.transpose(dst,
                                        x_sb[i][:, k * 128:(k + 1) * 128],
                                        ident_sb[:])
                for k in range(KCH):
                    src = pta[:, k * 128:(k + 1) * 128] if k < 4 \
                        else ptb[:, (k - 4) * 128:(k - 3) * 128]
                    if (i * KCH + k) % 2 == 0:
                        nc.scalar.activation(xt[k][:, i * 128:(i + 1) * 128],
                                             src, AF.Copy)
                    else:
                        nc.vector.tensor_copy(xt[k][:, i * 128:(i + 1) * 128],
                                              src)

            for i in range(4):
                emit_transpose_chunk(i)

            # ---- pos -> posT -> sin128 (signed) / cos128
            posT = misc.tile([64, N], F32R, name="posT", tag="posT")
            sin128 = misc.tile([128, N], F32, name="sin128", tag="sin128")
            cos128 = misc.tile([128, N], F32, name="cos128", tag="cos128")

            def emit_sincos():
                for half in range(2):
                    pp = ps.tile([64, 512], F32, name=f"pp{half}", tag="acc")
                    for i2 in range(4):
                        i = half * 4 + i2
                        nc.tensor.transpose(pp[:, i2 * 128:(i2 + 1) * 128],
                                            p_all[:, i * 64:(i + 1) * 64],
                                            ident_sb[:])
                    sl = slice(half * 512, (half + 1) * 512)
                    nc.scalar.activation(posT[:, sl], pp[:], AF.Copy)
                for half in range(2):
                    sl = slice(half * 512, (half + 1) * 512)
                    ps_s = ps.tile([128, 512], F32, name=f"ps_s{half}", tag="acc")
                    nc.tensor.matmul(ps_s[:], rs_sb[:], posT[:, sl],
                                     start=True, stop=True)
                    nc.vector.tensor_copy(sin128[:, sl], ps_s[:])
                    ps_c = ps.tile([128, 512], F32, name=f"ps_c{half}", tag="acc")
                    nc.tensor.matmul(ps_c[:], rc_sb[:], posT[:, sl],
                                     start=True, stop=True)
                    nc.vector.tensor_copy(cos128[:, sl], ps_c[:])

            # ---- helpers ----------------------------------------------------
            qk_tiles = {}
            qk_parts = {}

            def emit_qk_half(t, which, half):
                """One n-half of rotated q/k production for pair t."""
                cbase = t * 128 if which == "qA" else INNER + t * 128
                sl = slice(half * 512, (half + 1) * 512)
                if (t, which) not in qk_parts:
                    qs = qpool.tile([128, N], F32, name=f"qs_{which}_{t}",
                                    tag="qs", bufs=2)
                    qsw = qpool.tile([128, N], F32, name=f"qsw_{which}_{t}",
                                     tag="qsw", bufs=2)
                    qx = qpool.tile([128, N], F32, name=f"qx_{which}_{t}",
                                    tag=f"qx_{which}", bufs=1)
                    qf = qpool.tile([128, N], BF16, name=f"{which}_{t}",
                                    tag=which, bufs=2)
                    qk_parts[(t, which)] = (qs, qsw, qx, qf)
                    qk_tiles[(t, which)] = qf
                qs, qsw, qx, qf = qk_parts[(t, which)]
                acc = ps.tile([128, 512], F32, name=f"qk_{which}_{t}_{half}",
                              tag="acc")
                for k in range(KCH):
                    nc.tensor.matmul(acc[:], wqkv_r[k][:, cbase:cbase + 128],
                                     xt[k][:, sl],
                                     start=(k == 0), stop=(k == KCH - 1))
                nc.vector.tensor_mul(qs[:, sl], acc[:], sin128[:, sl])
                nc.vector.tensor_mul(qx[:, sl], acc[:], cos128[:, sl])
                if half == 1:
                    # pair-swap partitions on the full tile (sliced
                    # partition-split DMA APs mis-lower); one per queue
                    qs_v = qs[:].rearrange("(a two) n -> two a n", two=2)
                    qsw_v = qsw[:].rearrange("(a two) n -> two a n", two=2)
                    nc.sync.dma_start(qsw_v[0], qs_v[1])
                    nc.gpsimd.dma_start(qsw_v[1], qs_v[0])
                    for h2 in range(2):
                        s2 = slice(h2 * 512, (h2 + 1) * 512)
                        # f32 + f32 -> bf16 (single end-stage rounding)
                        nc.gpsimd.tensor_add(qf[:, s2], qx[:, s2], qsw[:, s2])

            def emit_qk(t):
                for which in ("qA", "qB"):
                    for half in range(2):
                        emit_qk_half(t, which, half)

            def emit_v(jc):
                """V for n-chunk jc into ones-augmented vaug[jc] (bf16)."""
                for half in range(2):
                    vp = ps.tile([128, 512], F32, name=f"vp_{jc}_{half}",
                                 tag="acc")
                    for k in range(KCH):
                        nc.tensor.matmul(
                            vp[:, 0:384],
                            xt[k][:, jc * 128:(jc + 1) * 128],
                            wqkv_r[k][:, 2 * INNER + half * 384:
                                      2 * INNER + (half + 1) * 384],
                            start=(k == 0), stop=(k == KCH - 1))
                    dst = vaug[jc][:, half * 390:half * 390 + 390] \
                        .rearrange("p (h d) -> p h d", d=65)[:, :, 0:64]
                    src = vp[:, 0:384].rearrange("p (h d) -> p h d", d=64)
                    nc.vector.tensor_copy(dst, src)

            ao = [big.tile([128, N], BF16, name=f"ao{t}", tag=f"ao{t}")
                  for t in range(TCH)]

            def emit_normalize(t, avsb):
                """Divide AV by denominators (row 64) into bf16 ao[t].

                Batch all 4 denominators through one [128, 16] reciprocal,
                bounce through DRAM, ONE broadcast read back [64, 2048];
                the divides run on GpSimd (all-SBUF) so the rep-DMA latency
                cannot head-of-line-block the Vector FIFO."""
                order = ((0, 0), (1, 0), (0, 1), (1, 1))
                dsq = npool.tile([128, 16], F32, name=f"dsq_{t}", tag="dsq")
                for c, (h, nh) in enumerate(order):
                    eng = nc.sync if c % 2 == 0 else nc.gpsimd
                    eng.dma_start(dsq[:, 4 * c:4 * (c + 1)],
                                  avsb[h][nh][64:65, :])
                nc.vector.reciprocal(dsq[:], dsq[:])
                # den writes + rep read on the SAME queue (gpsimd) so SWDGE
                # FIFO order guarantees write-before-read in DRAM
                for c in range(4):
                    nc.gpsimd.dma_start(den_d.ap()[t][c],
                                        dsq[:, 4 * c:4 * (c + 1)])
                rep = npool.tile([64, 4 * 512], F32, name=f"rep_{t}", tag="rep",
                                 bufs=1)
                nc.gpsimd.dma_start(
                    rep[:],
                    den_d.ap()[t].rearrange("c p q -> (c p q)")
                    .partition_broadcast(64))
                for c, (h, nh) in enumerate(order):
                    nc.gpsimd.tensor_mul(
                        ao[t][h * 64:(h + 1) * 64, nh * 512:(nh + 1) * 512],
                        avsb[h][nh][0:64, :],
                        rep[:, c * 512:(c + 1) * 512])

            def emit_attention(t, interleave):
                """Attention for pair t, one query-half (nh) at a time.
                interleave: dict (nh, jc) -> [callables]."""
                qa = qk_tiles[(t, "qA")]
                qb = qk_tiles[(t, "qB")]
                avsb = [[None, None], [None, None]]
                for nh in range(2):
                    sl = slice(nh * 512, (nh + 1) * 512)
                    av = [None, None]
                    for jc in range(NCH):
                        jsl = slice(jc * 128, (jc + 1) * 128)
                        stp0 = ps.tile([128, 512], F32,
                                       name=f"stp0_{t}_{nh}_{jc}", tag="stp",
                                       bufs=4)
                        stp1 = ps.tile([128, 512], F32,
                                       name=f"stp1_{t}_{nh}_{jc}", tag="stp",
                                       bufs=4)
                        nc.tensor.matmul(stp0[:], qb[0:64, jsl], qa[0:64, sl],
                                         start=True, stop=True,
                                         tile_position=(0, 0))
                        nc.tensor.matmul(stp1[:], qb[64:128, jsl],
                                         qa[64:128, sl],
                                         start=True, stop=True,
                                         tile_position=(64, 0))
                        e0 = epool.tile([128, 512], BF16, name=f"e0_{t}_{nh}_{jc}",
                                        tag="e", bufs=4)
                        nc.scalar.activation(e0[:], stp0[:], AF.Exp, scale=SCALE)
                        e1 = epool.tile([128, 512], BF16, name=f"e1_{t}_{nh}_{jc}",
                                        tag="e", bufs=4)
                        nc.scalar.activation(e1[:], stp1[:], AF.Exp, scale=SCALE)
                        if jc == 0:
                            for h in range(2):
                                av[h] = psv.tile([65, 512], F32,
                                                 name=f"av_{t}_{nh}_{h}",
                                                 tag="avs", bufs=2)
                        for h, e in ((0, e0), (1, e1)):
                            vsl = vaug[jc][:, (2 * t + h) * 65:
                                           (2 * t + h + 1) * 65]
                            nc.tensor.matmul(av[h][:], vsl, e[:],
                                             start=(jc == 0),
                                             stop=(jc == NCH - 1))
                        for fn in interleave.get((nh, jc), ()):
                            fn()
                    for h in range(2):
                        sb = npool.tile([65, 512], F32, name=f"avsb_{t}_{h}_{nh}",
                                        tag="avsb", bufs=4)
                        nc.vector.tensor_copy(sb[:], av[h][:])
                        avsb[h][nh] = sb
                emit_normalize(t, avsb)

            # ---- schedule ---------------------------------------------------
            emit_transpose_chunk(4)
            emit_transpose_chunk(5)
            emit_transpose_chunk(6)
            emit_transpose_chunk(7)
            emit_sincos()
            emit_qk_half(0, "qA", 0)
            emit_qk_half(0, "qB", 0)
            emit_v(0)
            emit_qk_half(0, "qA", 1)
            emit_qk_half(0, "qB", 1)
            emit_qk(1)

            for t in range(TCH):
                inter = {}
                if t == 0:
                    inter = {(0, jc): [lambda jc=jc: emit_v(jc + 1)]
                             for jc in range(7)}
                elif t < TCH - 1:
                    tn = t + 1
                    inter = {(0, 1): [lambda tn=tn: emit_qk_half(tn, "qA", 0)],
                             (0, 5): [lambda tn=tn: emit_qk_half(tn, "qA", 1)],
                             (1, 1): [lambda tn=tn: emit_qk_half(tn, "qB", 0)],
                             (1, 5): [lambda tn=tn: emit_qk_half(tn, "qB", 1)]}
                emit_attention(t, inter)

            # ---- output projection + bias
            for i in range(NCH):
                op_lo = ps.tile([128, 512], F32, name=f"oplo_{i}", tag="acc")
                op_hi = ps.tile([128, 256], F32, name=f"ophi_{i}", tag="acc")
                for k in range(KCH):
                    lhs = ao[k][:, i * 128:(i + 1) * 128]
                    nc.tensor.matmul(op_lo[:], lhs, wout_b[k][:, 0:512],
                                     start=(k == 0), stop=(k == KCH - 1))
                    nc.tensor.matmul(op_hi[:], lhs, wout_b[k][:, 512:768],
                                     start=(k == 0), stop=(k == KCH - 1))
                y_sb = tp.tile([128, DIM], F32, name=f"y_sb_{i}", tag="ysb", bufs=2)
                nc.vector.tensor_add(y_sb[:, 0:512], op_lo[:], b_bcast[:, 0:512])
                nc.vector.tensor_add(y_sb[:, 512:768], op_hi[:],
                                     b_bcast[:, 512:768])
                nc.sync.dma_start(y_d[i * 128:(i + 1) * 128, :], y_sb[:])

    nc.compile()
    return nc


def get_nc():
    if 'nc' not in _CACHE:
        _CACHE['nc'] = _build()
    return _CACHE['nc']


def make_in_maps(inputs):
    x = np.ascontiguousarray(np.asarray(inputs["x"], dtype=np.float32))
    pos = np.ascontiguousarray(
        np.asarray(inputs["pos_emb"], dtype=np.float32).reshape(N, DHEAD))
    wqkv = np.ascontiguousarray(np.asarray(inputs["W_qkv"], dtype=np.float32))
    wout = np.ascontiguousarray(np.asarray(inputs["W_out"], dtype=np.float32))
    bout = np.ascontiguousarray(np.asarray(inputs["b_out"], dtype=np.float32))
    return [{"x": np.ascontiguousarray(x[i]), "pos": pos, "wqkv": wqkv,
             "wout": wout, "bout": bout} for i in range(B)]


def run(inputs, trace=False, **kwargs):
    """inputs: dict with full-shape arrays as in reference.setup_inputs()."""
    from concourse.bass_utils import run_bass_kernel_spmd
    nc = get_nc()
    res = run_bass_kernel_spmd(nc, make_in_maps(inputs),
                               core_ids=list(range(B)), trace=trace, **kwargs)
    out = np.stack([res.results[i]["y"] for i in range(B)], axis=0)
    return out, res


def kernel(**inputs):
    out, _ = run(inputs, trace=False)
    return out


# revision 16
# speedup vs baseline: 1.0297x; 1.0297x over previous
"""Rotary multi-head attention (b=8, n=1024, dim=768, heads=12, d_head=64)
on 8 Trainium2 NeuronCores, data-parallel over batch (1 batch row per core).

v6: PSUM re-plan to decouple the score pipeline from qk production.
  - Attention runs per (pair, query-half): every PSUM tile is ONE bank.
    stp ring: 4x [128,512] (tag stp) -> scores run 2 steps ahead of exp.
    qk acc / vp / pt / op: own 2-slot tag "acc" -> DVE rotary muls no
    longer gate the score matmuls (v5's 3us LDW stalls).
    av: 2 live banks (per-nh lifetime), 2-slot ring.
  - Scalar runs only exp during attention ([128,512] granularity).
  - Rotated q/k kept f32 through the swap; single end-stage bf16 round
    into qf, which the bf16 score matmuls read (FWL weight loads).
  - Normalize: batched [128,16] reciprocal, DRAM bounce, one [64,2048]
    broadcast read, divide on GpSimd (keeps Vector FIFO clear).
"""
import sys
import numpy as np

if '/opt/trn_rl_repo' not in sys.path:
    sys.path.insert(0, '/opt/trn_rl_repo')

B, N, DIM = 8, 1024, 768
HEADS, DHEAD = 12, 64
INNER = HEADS * DHEAD           # 768
SCALE = DHEAD ** -0.5           # 0.125
NCH = N // 128                  # 8 n-chunks
KCH = DIM // 128                # 6 contraction chunks
TCH = HEADS // 2                # 6 head pairs

_CACHE = {}


def _build():
    import concourse.mybir as mybir
    from concourse import bacc
    from concourse.tile import TileContext

    F32 = mybir.dt.float32
    F32R = mybir.dt.float32r
    BF16 = mybir.dt.bfloat16
    AF = mybir.ActivationFunctionType

    nc = bacc.Bacc("TRN2", target_bir_lowering=False, debug=False, num_devices=8)

    x_d = nc.dram_tensor("x", [N, DIM], F32, kind="ExternalInput")
    pos_d = nc.dram_tensor("pos", [N, DHEAD], F32, kind="ExternalInput")
    wqkv_d = nc.dram_tensor("wqkv", [DIM, 3 * INNER], F32R, kind="ExternalInput")
    wout_d = nc.dram_tensor("wout", [INNER, DIM], F32, kind="ExternalInput")
    bout_d = nc.dram_tensor("bout", [DIM], F32, kind="ExternalInput")
    y_d = nc.dram_tensor("y", [N, DIM], F32, kind="ExternalOutput")
    den_d = nc.dram_tensor("den_scr", [TCH, 4, 128, 4], F32)

    # ---- inline constants -------------------------------------------------
    ident_d = nc.inline_tensor(np.eye(128, dtype=np.float32), name="ident")
    # sin table with pair-swap sign baked in: odd rows get -sin.
    # posT rows 0:32 = sin values, 32:64 = cos values.
    RsinT = np.zeros((64, 128), np.float32)
    RcosT = np.zeros((64, 128), np.float32)
    for m in range(128):
        RsinT[(m % 64) // 2, m] = -1.0 if (m % 2 == 1) else 1.0
        RcosT[32 + (m % 64) // 2, m] = 1.0
    rsinT_d = nc.inline_tensor(RsinT, name="rsinT")
    rcosT_d = nc.inline_tensor(RcosT, name="rcosT")

    with TileContext(nc) as tc:
        with tc.tile_pool(name="wp", bufs=1) as wp, \
             tc.tile_pool(name="big", bufs=1) as big, \
             tc.tile_pool(name="tp", bufs=2) as tp, \
             tc.tile_pool(name="qpool", bufs=2) as qpool, \
             tc.tile_pool(name="epool", bufs=1) as epool, \
             tc.tile_pool(name="npool", bufs=2) as npool, \
             tc.tile_pool(name="misc", bufs=1) as misc, \
             tc.tile_pool(name="ps", bufs=2, space="PSUM") as ps, \
             tc.tile_pool(name="psv", bufs=2, space="PSUM") as psv:

            # ---- input DMAs.  sync queue: x, consts, Q+Klo weight half.
            # gpsimd queue: pos, rotary tables, Khi+V weight half, wout.
            ident_sb = misc.tile([128, 128], F32, name="ident_sb", tag="ident_sb")
            nc.sync.dma_start(ident_sb[:], ident_d.ap())
            b_row = misc.tile([1, DIM], F32, name="b_row", tag="b_row")
            nc.sync.dma_start(b_row[:], bout_d.ap().unsqueeze(0))
            x_sb = []
            for i in range(NCH):
                xs = tp.tile([128, DIM], F32, name=f"x_sb_{i}", tag="xsb", bufs=3)
                nc.sync.dma_start(xs[:], x_d[i * 128:(i + 1) * 128, :])
                x_sb.append(xs)

            p_all = misc.tile([128, NCH * DHEAD], F32, name="p_all", tag="p_all")
            for i in range(NCH):
                nc.gpsimd.dma_start(p_all[:, i * DHEAD:(i + 1) * DHEAD],
                                    pos_d[i * 128:(i + 1) * 128, :])

            b_bcast = misc.tile([128, DIM], F32, name="b_bcast", tag="b_bcast")
            nc.gpsimd.partition_broadcast(b_bcast[:], b_row[:])

            rs_sb = misc.tile([64, 128], F32R, name="rs_sb", tag="rs_sb")
            nc.gpsimd.dma_start(rs_sb[:], rsinT_d.ap())
            rc_sb = misc.tile([64, 128], F32R, name="rc_sb", tag="rc_sb")
            nc.gpsimd.dma_start(rc_sb[:], rcosT_d.ap())

            wqkv_r = [wp.tile([128, 3 * INNER], F32R, name=f"wqkv_r_{k}",
                              tag=f"wqkv_r_{k}") for k in range(KCH)]
            half_cols = 3 * INNER // 2      # 1152
            for k in range(KCH):
                nc.sync.dma_start(wqkv_r[k][:, 0:half_cols],
                                  wqkv_d[k * 128:(k + 1) * 128, 0:half_cols])
                nc.gpsimd.dma_start(wqkv_r[k][:, half_cols:],
                                    wqkv_d[k * 128:(k + 1) * 128, half_cols:])
            # wout cast to bf16 by the software DGE
            wout_b = [wp.tile([128, DIM], BF16, name=f"wout_b_{k}",
                              tag=f"wout_b_{k}") for k in range(KCH)]
            for k in range(KCH):
                nc.gpsimd.dma_start(wout_b[k][:], wout_d[k * 128:(k + 1) * 128, :])

            # ---- vaug memset (ones columns give softmax denominators)
            vaug_all = big.tile([128, NCH * HEADS * 65], BF16, name="vaug",
                                tag="vaug")
            vaug = [vaug_all[:, i * HEADS * 65:(i + 1) * HEADS * 65]
                    for i in range(NCH)]
            nc.gpsimd.memset(vaug_all[:], 1.0)

            # preload the gpsimd tensor-tensor library off the critical path
            dummy = misc.tile([1, 16], F32, name="dummy_tt", tag="dummy_tt")
            nc.gpsimd.memset(dummy[:], 1.0)
            nc.gpsimd.tensor_add(dummy[:], dummy[:], dummy[:])

            # ---- X^T via PE transposes; PSUM->SBUF copies alternate
            # Scalar/Vector so neither serializes the transpose stream
            xt = [big.tile([128, N], F32R, name=f"xt{k}", tag=f"xt{k}")
                  for k in range(KCH)]

            def emit_transpose_chunk(i):
                pta = ps.tile([128, 512], F32, name=f"pta_{i}", tag="acc")
                ptb = ps.tile([128, 256], F32, name=f"ptb_{i}", tag="acc")
                for k in range(KCH):
                    dst = pta[:, k * 128:(k + 1) * 128] if k < 4 \
                        else ptb[:, (k - 4) * 128:(k - 3) * 128]
                    nc.tensor.transpose(dst,
                                        x_sb[i][:, k * 128:(k + 1) * 128],
                                        ident_sb[:])
                for k in range(KCH):
                    src = pta[:, k * 128:(k + 1) * 128] if k < 4 \
                        else ptb[:, (k - 4) * 128:(k - 3) * 128]
                    if (i * KCH + k) % 2 == 0:
                        nc.scalar.activation(xt[k][:, i * 128:(i + 1) * 128],
                                             src, AF.Copy)
                    else:
                        nc.vector.tensor_copy(xt[k][:, i * 128:(i + 1) * 128],
                                              src)

            for i in range(4):
                emit_transpose_chunk(i)

            # ---- pos -> posT -> sin128 (signed) / cos128
            posT = misc.tile([64, N], F32R, name="posT", tag="posT")
            sin128 = misc.tile([128, N], F32, name="sin128", tag="sin128")
            cos128 = misc.tile([128, N], F32, name="cos128", tag="cos128")

            def emit_sincos():
                for half in range(2):
                    pp = ps.tile([64, 512], F32, name=f"pp{half}", tag="acc")
                    for i2 in range(4):
                        i = half * 4 + i2
                        nc.tensor.transpose(pp[:, i2 * 128:(i2 + 1) * 128],
                                            p_all[:, i * 64:(i + 1) * 64],
                                            ident_sb[:])
                    sl = slice(half * 512, (half + 1) * 512)
                    nc.scalar.activation(posT[:, sl], pp[:], AF.Copy)
                for half in range(2):
                    sl = slice(half * 512, (half + 1) * 512)
                    ps_s = ps.tile([128, 512], F32, name=f"ps_s{half}", tag="acc")
                    nc.tensor.matmul(ps_s[:], rs_sb[:], posT[:, sl],
                                     start=True, stop=True)
                    nc.vector.tensor_copy(sin128[:, sl], ps_s[:])
                    ps_c = ps.tile([128, 512], F32, name=f"ps_c{half}", tag="acc")
                    nc.tensor.matmul(ps_c[:], rc_sb[:], posT[:, sl],
                                     start=True, stop=True)
                    nc.vector.tensor_copy(cos128[:, sl], ps_c[:])

            # ---- helpers ----------------------------------------------------
            qk_tiles = {}
            qk_parts = {}

            def emit_qk_half(t, which, half):
                """One n-half of rotated q/k production for pair t."""
                cbase = t * 128 if which == "qA" else INNER + t * 128
                sl = slice(half * 512, (half + 1) * 512)
                if (t, which) not in qk_parts:
                    qs = qpool.tile([128, N], F32, name=f"qs_{which}_{t}",
                                    tag="qs", bufs=2)
                    qsw = qpool.tile([128, N], F32, name=f"qsw_{which}_{t}",
                                     tag="qsw", bufs=2)
                    qx = qpool.tile([128, N], F32, name=f"qx_{which}_{t}",
                                    tag=f"qx_{which}", bufs=1)
                    qf = qpool.tile([128, N], BF16, name=f"{which}_{t}",
                                    tag=which, bufs=2)
                    qk_parts[(t, which)] = (qs, qsw, qx, qf)
                    qk_tiles[(t, which)] = qf
                qs, qsw, qx, qf = qk_parts[(t, which)]
                acc = ps.tile([128, 512], F32, name=f"qk_{which}_{t}_{half}",
                              tag="acc")
                for k in range(KCH):
                    nc.tensor.matmul(acc[:], wqkv_r[k][:, cbase:cbase + 128],
                                     xt[k][:, sl],
                                     start=(k == 0), stop=(k == KCH - 1))
                nc.vector.tensor_mul(qs[:, sl], acc[:], sin128[:, sl])
                nc.vector.tensor_mul(qx[:, sl], acc[:], cos128[:, sl])
                if half == 1:
                    # pair-swap partitions on the full tile (sliced
                    # partition-split DMA APs mis-lower); one per queue
                    qs_v = qs[:].rearrange("(a two) n -> two a n", two=2)
                    qsw_v = qsw[:].rearrange("(a two) n -> two a n", two=2)
                    nc.sync.dma_start(qsw_v[0], qs_v[1])
                    nc.gpsimd.dma_start(qsw_v[1], qs_v[0])
                    for h2 in range(2):
                        s2 = slice(h2 * 512, (h2 + 1) * 512)
                        # f32 + f32 -> bf16 (single end-stage rounding)
                        nc.gpsimd.tensor_add(qf[:, s2], qx[:, s2], qsw[:, s2])

            def emit_qk(t):
                for which in ("qA", "qB"):
                    for half in range(2):
                        emit_qk_half(t, which, half)

            def emit_v(jc):
                """V for n-chunk jc into ones-augmented vaug[jc] (bf16)."""
                for half in range(2):
                    vp = ps.tile([128, 512], F32, name=f"vp_{jc}_{half}",
                                 tag="acc")
                    for k in range(KCH):
                        nc.tensor.matmul(
                            vp[:, 0:384],
                            xt[k][:, jc * 128:(jc + 1) * 128],
                            wqkv_r[k][:, 2 * INNER + half * 384:
                                      2 * INNER + (half + 1) * 384],
                            start=(k == 0), stop=(k == KCH - 1))
                    dst = vaug[jc][:, half * 390:half * 390 + 390] \
                        .rearrange("p (h d) -> p h d", d=65)[:, :, 0:64]
                    src = vp[:, 0:384].rearrange("p (h d) -> p h d", d=64)
                    nc.vector.tensor_copy(dst, src)

            ao = [big.tile([128, N], BF16, name=f"ao{t}", tag=f"ao{t}")
                  for t in range(TCH)]

            def emit_normalize(t, avsb):
                """Divide AV by denominators (row 64) into bf16 ao[t].

                Batch all 4 denominators through one [128, 16] reciprocal,
                bounce through DRAM, ONE broadcast read back [64, 2048];
                the divides run on GpSimd (all-SBUF) so the rep-DMA latency
                cannot head-of-line-block the Vector FIFO."""
                order = ((0, 0), (1, 0), (0, 1), (1, 1))
                dsq = npool.tile([128, 16], F32, name=f"dsq_{t}", tag="dsq")
                for c, (h, nh) in enumerate(order):
                    eng = nc.sync if c % 2 == 0 else nc.gpsimd
                    eng.dma_start(dsq[:, 4 * c:4 * (c + 1)],
                                  avsb[h][nh][64:65, :])
                nc.vector.reciprocal(dsq[:], dsq[:])
                # den writes + rep read on the SAME queue (gpsimd) so SWDGE
                # FIFO order guarantees write-before-read in DRAM
                for c in range(4):
                    nc.gpsimd.dma_start(den_d.ap()[t][c],
                                        dsq[:, 4 * c:4 * (c + 1)])
                rep = npool.tile([64, 4 * 512], F32, name=f"rep_{t}", tag="rep",
                                 bufs=1)
                nc.gpsimd.dma_start(
                    rep[:],
                    den_d.ap()[t].rearrange("c p q -> (c p q)")
                    .partition_broadcast(64))
                for c, (h, nh) in enumerate(order):
                    nc.gpsimd.tensor_mul(
                        ao[t][h * 64:(h + 1) * 64, nh * 512:(nh + 1) * 512],
                        avsb[h][nh][0:64, :],
                        rep[:, c * 512:(c + 1) * 512])

            def emit_attention(t, interleave):
                """Attention for pair t, one query-half (nh) at a time.
                Scores/exp are EMITTED one step ahead of the AV matmuls so
                the PE FIFO never blocks on exp latency (software pipeline).
                interleave: dict (nh, step) -> [callables]."""
                qa = qk_tiles[(t, "qA")]
                qb = qk_tiles[(t, "qB")]
                avsb = [[None, None], [None, None]]
                for nh in range(2):
                    sl = slice(nh * 512, (nh + 1) * 512)
                    av = [None, None]
                    es = {}
                    for step in range(NCH + 1):
                        if step < NCH:
                            jc = step
                            jsl = slice(jc * 128, (jc + 1) * 128)
                            stp0 = ps.tile([128, 512], F32,
                                           name=f"stp0_{t}_{nh}_{jc}", tag="stp",
                                           bufs=4)
                            stp1 = ps.tile([128, 512], F32,
                                           name=f"stp1_{t}_{nh}_{jc}", tag="stp",
                                           bufs=4)
                            nc.tensor.matmul(stp0[:], qb[0:64, jsl],
                                             qa[0:64, sl],
                                             start=True, stop=True,
                                             tile_position=(0, 0))
                            nc.tensor.matmul(stp1[:], qb[64:128, jsl],
                                             qa[64:128, sl],
                                             start=True, stop=True,
                                             tile_position=(64, 0))
                            e0 = epool.tile([128, 512], BF16,
                                            name=f"e0_{t}_{nh}_{jc}",
                                            tag="e", bufs=6)
                            nc.scalar.activation(e0[:], stp0[:], AF.Exp,
                                                 scale=SCALE)
                            e1 = epool.tile([128, 512], BF16,
                                            name=f"e1_{t}_{nh}_{jc}",
                                            tag="e", bufs=6)
                            nc.scalar.activation(e1[:], stp1[:], AF.Exp,
                                                 scale=SCALE)
                            es[jc] = (e0, e1)
                            if jc == 0:
                                for h in range(2):
                                    av[h] = psv.tile([65, 512], F32,
                                                     name=f"av_{t}_{nh}_{h}",
                                                     tag="avs", bufs=2)
                        if step > 0:
                            jc = step - 1
                            e0, e1 = es.pop(jc)
                            for h, e in ((0, e0), (1, e1)):
                                vsl = vaug[jc][:, (2 * t + h) * 65:
                                               (2 * t + h + 1) * 65]
                                nc.tensor.matmul(av[h][:], vsl, e[:],
                                                 start=(jc == 0),
                                                 stop=(jc == NCH - 1))
                        for fn in interleave.get((nh, step), ()):
                            fn()
                    for h in range(2):
                        sb = npool.tile([65, 512], F32, name=f"avsb_{t}_{h}_{nh}",
                                        tag="avsb", bufs=4)
                        nc.vector.tensor_copy(sb[:], av[h][:])
                        avsb[h][nh] = sb
                emit_normalize(t, avsb)

            # ---- schedule ---------------------------------------------------
            emit_transpose_chunk(4)
            emit_transpose_chunk(5)
            emit_transpose_chunk(6)
            emit_transpose_chunk(7)
            emit_sincos()
            emit_qk_half(0, "qA", 0)
            emit_qk_half(0, "qB", 0)
            emit_v(0)
            emit_qk_half(0, "qA", 1)
            emit_qk_half(0, "qB", 1)
            emit_qk(1)

            for t in range(TCH):
                inter = {}
                if t == 0:
                    inter = {(0, jc): [lambda jc=jc: emit_v(jc + 1)]
                             for jc in range(7)}
                elif t < TCH - 1:
                    tn = t + 1
                    inter = {(0, 1): [lambda tn=tn: emit_qk_half(tn, "qA", 0)],
                             (0, 5): [lambda tn=tn: emit_qk_half(tn, "qA", 1)],
                             (1, 1): [lambda tn=tn: emit_qk_half(tn, "qB", 0)],
                             (1, 5): [lambda tn=tn: emit_qk_half(tn, "qB", 1)]}
                emit_attention(t, inter)

            # ---- output projection + bias
            for i in range(NCH):
                op_lo = ps.tile([128, 512], F32, name=f"oplo_{i}", tag="acc")
                op_hi = ps.tile([128, 256], F32, name=f"ophi_{i}", tag="acc")
                for k in range(KCH):
                    lhs = ao[k][:, i * 128:(i + 1) * 128]
                    nc.tensor.matmul(op_lo[:], lhs, wout_b[k][:, 0:512],
                                     start=(k == 0), stop=(k == KCH - 1))
                    nc.tensor.matmul(op_hi[:], lhs, wout_b[k][:, 512:768],
                                     start=(k == 0), stop=(k == KCH - 1))
                y_sb = tp.tile([128, DIM], F32, name=f"y_sb_{i}", tag="ysb", bufs=2)
                nc.vector.tensor_add(y_sb[:, 0:512], op_lo[:], b_bcast[:, 0:512])
                nc.vector.tensor_add(y_sb[:, 512:768], op_hi[:],
                                     b_bcast[:, 512:768])
                nc.sync.dma_start(y_d[i * 128:(i + 1) * 128, :], y_sb[:])

    nc.compile()
    return nc


def get_nc():
    if 'nc' not in _CACHE:
        _CACHE['nc'] = _build()
    return _CACHE['nc']


def make_in_maps(inputs):
    x = np.ascontiguousarray(np.asarray(inputs["x"], dtype=np.float32))
    pos = np.ascontiguousarray(
        np.asarray(inputs["pos_emb"], dtype=np.float32).reshape(N, DHEAD))
    wqkv = np.ascontiguousarray(np.asarray(inputs["W_qkv"], dtype=np.float32))
    wout = np.ascontiguousarray(np.asarray(inputs["W_out"], dtype=np.float32))
    bout = np.ascontiguousarray(np.asarray(inputs["b_out"], dtype=np.float32))
    return [{"x": np.ascontiguousarray(x[i]), "pos": pos, "wqkv": wqkv,
             "wout": wout, "bout": bout} for i in range(B)]


def run(inputs, trace=False, **kwargs):
    """inputs: dict with full-shape arrays as in reference.setup_inputs()."""
    from concourse.bass_utils import run_bass_kernel_spmd
    nc = get_nc()
    res = run_bass_kernel_spmd(nc, make_in_maps(inputs),
                               core_ids=list(range(B)), trace=trace, **kwargs)
    out = np.stack([res.results[i]["y"] for i in range(B)], axis=0)
    return out, res


def kernel(**inputs):
    out, _ = run(inputs, trace=False)
    return out


# revision 17
# speedup vs baseline: 1.1448x; 1.1118x over previous
"""Rotary multi-head attention (b=8, n=1024, dim=768, heads=12, d_head=64)
on 8 Trainium2 NeuronCores, data-parallel over batch (1 batch row per core).

v8: coarse exp granularity + software-pipelined AV + paired PSUM parity.
  - exp at [128,1024] (96 ACTIVATEs): fine 512-granularity (v7) paid
    ~45% per-instruction Scalar overhead and ACT-paced the kernel.
  - Scores/exp EMITTED one step ahead of AV matmuls: the strict-FIFO PE
    queue never blocks on exp latency.
  - All non-stp PSUM users (qk acc, vp, pt, pp, sincos, op) allocate in
    ADJACENT PAIRS from the shared 2-slot "mm" tag, preserving rotation
    parity so score tiles never land on a slot gated by DVE rotary muls
    (v5's 3us LDWEIGHTS stalls).
  - Scalar runs only exp during attention (avsb copies on Vector, final
    divides on GpSimd, sin/cos copies on Vector).
  - Rotated q/k kept f32 through the swap; single end-stage bf16 round
    into qf; score matmuls read bf16 (FWL).
  - Normalize: batched [128,16] reciprocal, DRAM bounce, one [64,2048]
    broadcast read; den writes + rep read on one queue (FIFO ordering).
"""
import sys
import numpy as np

if '/opt/trn_rl_repo' not in sys.path:
    sys.path.insert(0, '/opt/trn_rl_repo')

B, N, DIM = 8, 1024, 768
HEADS, DHEAD = 12, 64
INNER = HEADS * DHEAD           # 768
SCALE = DHEAD ** -0.5           # 0.125
NCH = N // 128                  # 8 n-chunks
KCH = DIM // 128                # 6 contraction chunks
TCH = HEADS // 2                # 6 head pairs

_CACHE = {}


def _build():
    import concourse.mybir as mybir
    from concourse import bacc
    from concourse.tile import TileContext

    F32 = mybir.dt.float32
    F32R = mybir.dt.float32r
    BF16 = mybir.dt.bfloat16
    AF = mybir.ActivationFunctionType

    nc = bacc.Bacc("TRN2", target_bir_lowering=False, debug=False, num_devices=8)

    x_d = nc.dram_tensor("x", [N, DIM], F32, kind="ExternalInput")
    pos_d = nc.dram_tensor("pos", [N, DHEAD], F32, kind="ExternalInput")
    wqkv_d = nc.dram_tensor("wqkv", [DIM, 3 * INNER], F32R, kind="ExternalInput")
    wout_d = nc.dram_tensor("wout", [INNER, DIM], F32, kind="ExternalInput")
    bout_d = nc.dram_tensor("bout", [DIM], F32, kind="ExternalInput")
    y_d = nc.dram_tensor("y", [N, DIM], F32, kind="ExternalOutput")
    den_d = nc.dram_tensor("den_scr", [TCH, 4, 128, 4], F32)

    # ---- inline constants -------------------------------------------------
    ident_d = nc.inline_tensor(np.eye(128, dtype=np.float32), name="ident")
    # sin table with pair-swap sign baked in: odd rows get -sin.
    # posT rows 0:32 = sin values, 32:64 = cos values.
    RsinT = np.zeros((64, 128), np.float32)
    RcosT = np.zeros((64, 128), np.float32)
    for m in range(128):
        RsinT[(m % 64) // 2, m] = -1.0 if (m % 2 == 1) else 1.0
        RcosT[32 + (m % 64) // 2, m] = 1.0
    rsinT_d = nc.inline_tensor(RsinT, name="rsinT")
    rcosT_d = nc.inline_tensor(RcosT, name="rcosT")

    with TileContext(nc) as tc:
        with tc.tile_pool(name="wp", bufs=1) as wp, \
             tc.tile_pool(name="big", bufs=1) as big, \
             tc.tile_pool(name="tp", bufs=2) as tp, \
             tc.tile_pool(name="qpool", bufs=2) as qpool, \
             tc.tile_pool(name="epool", bufs=1) as epool, \
             tc.tile_pool(name="npool", bufs=2) as npool, \
             tc.tile_pool(name="misc", bufs=1) as misc, \
             tc.tile_pool(name="ps", bufs=2, space="PSUM") as ps, \
             tc.tile_pool(name="psv", bufs=4, space="PSUM") as psv:

            # ---- input DMAs.  sync queue: x, consts, Q+Klo weight half.
            # gpsimd queue: pos, rotary tables, Khi+V weight half, wout.
            ident_sb = misc.tile([128, 128], F32, name="ident_sb", tag="ident_sb")
            nc.sync.dma_start(ident_sb[:], ident_d.ap())
            b_row = misc.tile([1, DIM], F32, name="b_row", tag="b_row")
            nc.sync.dma_start(b_row[:], bout_d.ap().unsqueeze(0))
            x_sb = []
            for i in range(NCH):
                xs = tp.tile([128, DIM], F32, name=f"x_sb_{i}", tag="xsb", bufs=3)
                nc.sync.dma_start(xs[:], x_d[i * 128:(i + 1) * 128, :])
                x_sb.append(xs)

            p_all = misc.tile([128, NCH * DHEAD], F32, name="p_all", tag="p_all")
            for i in range(NCH):
                nc.gpsimd.dma_start(p_all[:, i * DHEAD:(i + 1) * DHEAD],
                                    pos_d[i * 128:(i + 1) * 128, :])

            b_bcast = misc.tile([128, DIM], F32, name="b_bcast", tag="b_bcast")
            nc.gpsimd.partition_broadcast(b_bcast[:], b_row[:])

            rs_sb = misc.tile([64, 128], F32R, name="rs_sb", tag="rs_sb")
            nc.gpsimd.dma_start(rs_sb[:], rsinT_d.ap())
            rc_sb = misc.tile([64, 128], F32R, name="rc_sb", tag="rc_sb")
            nc.gpsimd.dma_start(rc_sb[:], rcosT_d.ap())

            wqkv_r = [wp.tile([128, 3 * INNER], F32R, name=f"wqkv_r_{k}",
                              tag=f"wqkv_r_{k}") for k in range(KCH)]
            half_cols = 3 * INNER // 2      # 1152
            for k in range(KCH):
                nc.sync.dma_start(wqkv_r[k][:, 0:half_cols],
                                  wqkv_d[k * 128:(k + 1) * 128, 0:half_cols])
                nc.gpsimd.dma_start(wqkv_r[k][:, half_cols:],
                                    wqkv_d[k * 128:(k + 1) * 128, half_cols:])
            # wout cast to bf16 by the software DGE
            wout_b = [wp.tile([128, DIM], BF16, name=f"wout_b_{k}",
                              tag=f"wout_b_{k}") for k in range(KCH)]
            for k in range(KCH):
                nc.gpsimd.dma_start(wout_b[k][:], wout_d[k * 128:(k + 1) * 128, :])

            # ---- vaug memset (ones columns give softmax denominators)
            vaug_all = big.tile([128, NCH * HEADS * 65], BF16, name="vaug",
                                tag="vaug")
            vaug = [vaug_all[:, i * HEADS * 65:(i + 1) * HEADS * 65]
                    for i in range(NCH)]
            nc.gpsimd.memset(vaug_all[:], 1.0)

            # preload the gpsimd tensor-tensor library off the critical path
            dummy = misc.tile([1, 16], F32, name="dummy_tt", tag="dummy_tt")
            nc.gpsimd.memset(dummy[:], 1.0)
            nc.gpsimd.tensor_add(dummy[:], dummy[:], dummy[:])

            # ---- X^T via PE transposes; PSUM->SBUF copies alternate
            # Scalar/Vector.  pt emitted as an adjacent PAIR (parity).
            xt = [big.tile([128, N], F32R, name=f"xt{k}", tag=f"xt{k}")
                  for k in range(KCH)]

            def emit_transpose_chunk(i):
                pta = ps.tile([128, 512], F32, name=f"pta_{i}", tag="mm")
                ptb = ps.tile([128, 512], F32, name=f"ptb_{i}", tag="mm")
                for k in range(KCH):
                    dst = pta[:, k * 128:(k + 1) * 128] if k < 4 \
                        else ptb[:, (k - 4) * 128:(k - 3) * 128]
                    nc.tensor.transpose(dst,
                                        x_sb[i][:, k * 128:(k + 1) * 128],
                                        ident_sb[:])
                for k in range(KCH):
                    src = pta[:, k * 128:(k + 1) * 128] if k < 4 \
                        else ptb[:, (k - 4) * 128:(k - 3) * 128]
                    if (i * KCH + k) % 2 == 0:
                        nc.scalar.activation(xt[k][:, i * 128:(i + 1) * 128],
                                             src, AF.Copy)
                    else:
                        nc.vector.tensor_copy(xt[k][:, i * 128:(i + 1) * 128],
                                              src)

            for i in range(4):
                emit_transpose_chunk(i)

            # ---- pos -> posT -> sin128 (signed) / cos128
            posT = misc.tile([64, N], F32R, name="posT", tag="posT")
            sin128 = misc.tile([128, N], F32, name="sin128", tag="sin128")
            cos128 = misc.tile([128, N], F32, name="cos128", tag="cos128")

            def emit_sincos():
                pps = []
                for half in range(2):
                    pp = ps.tile([64, 512], F32, name=f"pp{half}", tag="mm")
                    pps.append(pp)
                    for i2 in range(4):
                        i = half * 4 + i2
                        nc.tensor.transpose(pp[:, i2 * 128:(i2 + 1) * 128],
                                            p_all[:, i * 64:(i + 1) * 64],
                                            ident_sb[:])
                for half in range(2):
                    sl = slice(half * 512, (half + 1) * 512)
                    nc.scalar.activation(posT[:, sl], pps[half][:], AF.Copy)
                for half in range(2):
                    sl = slice(half * 512, (half + 1) * 512)
                    ps_s = ps.tile([128, 512], F32, name=f"ps_s{half}", tag="mm")
                    nc.tensor.matmul(ps_s[:], rs_sb[:], posT[:, sl],
                                     start=True, stop=True)
                    ps_c = ps.tile([128, 512], F32, name=f"ps_c{half}", tag="mm")
                    nc.tensor.matmul(ps_c[:], rc_sb[:], posT[:, sl],
                                     start=True, stop=True)
                    nc.vector.tensor_copy(sin128[:, sl], ps_s[:])
                    nc.vector.tensor_copy(cos128[:, sl], ps_c[:])

            # ---- helpers ----------------------------------------------------
            qk_tiles = {}
            qk_parts = {}

            def emit_qk_half(t, which, half):
                """One n-half of rotated q/k production for pair t."""
                cbase = t * 128 if which == "qA" else INNER + t * 128
                sl = slice(half * 512, (half + 1) * 512)
                if (t, which) not in qk_parts:
                    qs = qpool.tile([128, N], F32, name=f"qs_{which}_{t}",
                                    tag="qs", bufs=2)
                    qsw = qpool.tile([128, N], F32, name=f"qsw_{which}_{t}",
                                     tag="qsw", bufs=2)
                    qx = qpool.tile([128, N], F32, name=f"qx_{which}_{t}",
                                    tag=f"qx_{which}", bufs=1)
                    qf = qpool.tile([128, N], BF16, name=f"{which}_{t}",
                                    tag=which, bufs=2)
                    qk_parts[(t, which)] = (qs, qsw, qx, qf)
                    qk_tiles[(t, which)] = qf
                qs, qsw, qx, qf = qk_parts[(t, which)]
                acc = ps.tile([128, 512], F32, name=f"qk_{which}_{t}_{half}",
                              tag="mm")
                for k in range(KCH):
                    nc.tensor.matmul(acc[:], wqkv_r[k][:, cbase:cbase + 128],
                                     xt[k][:, sl],
                                     start=(k == 0), stop=(k == KCH - 1))
                nc.vector.tensor_mul(qs[:, sl], acc[:], sin128[:, sl])
                nc.vector.tensor_mul(qx[:, sl], acc[:], cos128[:, sl])
                if half == 1:
                    # pair-swap partitions on the full tile (sliced
                    # partition-split DMA APs mis-lower); one per queue
                    qs_v = qs[:].rearrange("(a two) n -> two a n", two=2)
                    qsw_v = qsw[:].rearrange("(a two) n -> two a n", two=2)
                    nc.sync.dma_start(qsw_v[0], qs_v[1])
                    nc.gpsimd.dma_start(qsw_v[1], qs_v[0])
                    for h2 in range(2):
                        s2 = slice(h2 * 512, (h2 + 1) * 512)
                        # f32 + f32 -> bf16 (single end-stage rounding)
                        nc.gpsimd.tensor_add(qf[:, s2], qx[:, s2], qsw[:, s2])

            def emit_qk_prod(t, which):
                """Both halves back-to-back: 2 adjacent 'mm' allocations."""
                emit_qk_half(t, which, 0)
                emit_qk_half(t, which, 1)

            def emit_v(jc):
                """V for n-chunk jc into ones-augmented vaug[jc] (bf16).
                Two adjacent 'mm' allocations (parity-preserving)."""
                for half in range(2):
                    vp = ps.tile([128, 512], F32, name=f"vp_{jc}_{half}",
                                 tag="mm")
                    for k in range(KCH):
                        nc.tensor.matmul(
                            vp[:, 0:384],
                            xt[k][:, jc * 128:(jc + 1) * 128],
                            wqkv_r[k][:, 2 * INNER + half * 384:
                                      2 * INNER + (half + 1) * 384],
                            start=(k == 0), stop=(k == KCH - 1))
                    dst = vaug[jc][:, half * 390:half * 390 + 390] \
                        .rearrange("p (h d) -> p h d", d=65)[:, :, 0:64]
                    src = vp[:, 0:384].rearrange("p (h d) -> p h d", d=64)
                    nc.vector.tensor_copy(dst, src)

            ao = [big.tile([128, N], BF16, name=f"ao{t}", tag=f"ao{t}")
                  for t in range(TCH)]

            def emit_normalize(t, avsb):
                """Divide AV by denominators (row 64) into bf16 ao[t]."""
                order = ((0, 0), (1, 0), (0, 1), (1, 1))
                dsq = npool.tile([128, 16], F32, name=f"dsq_{t}", tag="dsq")
                for c, (h, nh) in enumerate(order):
                    eng = nc.sync if c % 2 == 0 else nc.gpsimd
                    eng.dma_start(dsq[:, 4 * c:4 * (c + 1)],
                                  avsb[h][nh][64:65, :])
                nc.vector.reciprocal(dsq[:], dsq[:])
                # den writes + rep read on the SAME queue (gpsimd) so SWDGE
                # FIFO order guarantees write-before-read in DRAM
                for c in range(4):
                    nc.gpsimd.dma_start(den_d.ap()[t][c],
                                        dsq[:, 4 * c:4 * (c + 1)])
                rep = npool.tile([64, 4 * 512], F32, name=f"rep_{t}", tag="rep",
                                 bufs=1)
                nc.gpsimd.dma_start(
                    rep[:],
                    den_d.ap()[t].rearrange("c p q -> (c p q)")
                    .partition_broadcast(64))
                # divides on GpSimd (all-SBUF): keeps the Vector FIFO clear
                for c, (h, nh) in enumerate(order):
                    nc.gpsimd.tensor_mul(
                        ao[t][h * 64:(h + 1) * 64, nh * 512:(nh + 1) * 512],
                        avsb[h][nh][0:64, :],
                        rep[:, c * 512:(c + 1) * 512])

            def emit_attention(t, interleave):
                """Attention for pair t.  Scores/exp emitted one step ahead
                of the AV matmuls (software pipeline over jc).
                interleave: dict step -> [callables]."""
                qa = qk_tiles[(t, "qA")]
                qb = qk_tiles[(t, "qB")]
                avsb = [[None, None], [None, None]]
                av = [[None, None], [None, None]]
                es = {}
                for step in range(NCH + 1):
                    if step < NCH:
                        jc = step
                        jsl = slice(jc * 128, (jc + 1) * 128)
                        stp0 = ps.tile([128, N], F32, name=f"stp0_{t}_{jc}",
                                       tag="mm")
                        stp1 = ps.tile([128, N], F32, name=f"stp1_{t}_{jc}",
                                       tag="mm")
                        for hf in range(2):
                            sl = slice(hf * 512, (hf + 1) * 512)
                            nc.tensor.matmul(stp0[:, sl], qb[0:64, jsl],
                                             qa[0:64, sl],
                                             start=True, stop=True,
                                             tile_position=(0, 0))
                            nc.tensor.matmul(stp1[:, sl], qb[64:128, jsl],
                                             qa[64:128, sl],
                                             start=True, stop=True,
                                             tile_position=(64, 0))
                        e0 = epool.tile([128, N], BF16, name=f"e0_{t}_{jc}",
                                        tag="e", bufs=4)
                        nc.scalar.activation(e0[:], stp0[:], AF.Exp, scale=SCALE)
                        e1 = epool.tile([128, N], BF16, name=f"e1_{t}_{jc}",
                                        tag="e", bufs=4)
                        nc.scalar.activation(e1[:], stp1[:], AF.Exp, scale=SCALE)
                        es[jc] = (e0, e1)
                        if jc == 0:
                            for h in range(2):
                                for nh in range(2):
                                    av[h][nh] = psv.tile(
                                        [65, 512], F32, name=f"av_{t}_{h}_{nh}",
                                        tag="avs")
                    if step > 0:
                        jc = step - 1
                        e0, e1 = es.pop(jc)
                        for h, e in ((0, e0), (1, e1)):
                            vsl = vaug[jc][:, (2 * t + h) * 65:
                                           (2 * t + h + 1) * 65]
                            for nh in range(2):
                                nc.tensor.matmul(av[h][nh][:], vsl,
                                                 e[:, nh * 512:(nh + 1) * 512],
                                                 start=(jc == 0),
                                                 stop=(jc == NCH - 1))
                    for fn in interleave.get(step, ()):
                        fn()
                for h in range(2):
                    for nh in range(2):
                        sb = npool.tile([65, 512], F32, name=f"avsb_{t}_{h}_{nh}",
                                        tag="avsb", bufs=4)
                        nc.vector.tensor_copy(sb[:], av[h][nh][:])
                        avsb[h][nh] = sb
                emit_normalize(t, avsb)

            # ---- schedule ---------------------------------------------------
            emit_transpose_chunk(4)
            emit_transpose_chunk(5)
            emit_transpose_chunk(6)
            emit_transpose_chunk(7)
            emit_sincos()
            emit_qk_prod(0, "qA")
            emit_qk_prod(0, "qB")
            emit_v(0)
            emit_qk_prod(1, "qA")
            emit_qk_prod(1, "qB")

            for t in range(TCH):
                inter = {}
                if t == 0:
                    inter = {jc: [lambda jc=jc: emit_v(jc + 1)]
                             for jc in range(7)}
                elif t < TCH - 1:
                    tn = t + 1
                    inter = {2: [lambda tn=tn: emit_qk_prod(tn, "qA")],
                             5: [lambda tn=tn: emit_qk_prod(tn, "qB")]}
                emit_attention(t, inter)

            # ---- output projection + bias
            for i in range(NCH):
                op_lo = ps.tile([128, 512], F32, name=f"oplo_{i}", tag="mm")
                op_hi = ps.tile([128, 512], F32, name=f"ophi_{i}", tag="mm")
                for k in range(KCH):
                    lhs = ao[k][:, i * 128:(i + 1) * 128]
                    nc.tensor.matmul(op_lo[:], lhs, wout_b[k][:, 0:512],
                                     start=(k == 0), stop=(k == KCH - 1))
                    nc.tensor.matmul(op_hi[:, 0:256], lhs, wout_b[k][:, 512:768],
                                     start=(k == 0), stop=(k == KCH - 1))
                y_sb = tp.tile([128, DIM], F32, name=f"y_sb_{i}", tag="ysb", bufs=2)
                nc.vector.tensor_add(y_sb[:, 0:512], op_lo[:], b_bcast[:, 0:512])
                nc.vector.tensor_add(y_sb[:, 512:768], op_hi[:, 0:256],
                                     b_bcast[:, 512:768])
                nc.sync.dma_start(y_d[i * 128:(i + 1) * 128, :], y_sb[:])

    nc.compile()
    return nc


def get_nc():
    if 'nc' not in _CACHE:
        _CACHE['nc'] = _build()
    return _CACHE['nc']


def make_in_maps(inputs):
    x = np.ascontiguousarray(np.asarray(inputs["x"], dtype=np.float32))
    pos = np.ascontiguousarray(
        np.asarray(inputs["pos_emb"], dtype=np.float32).reshape(N, DHEAD))
    wqkv = np.ascontiguousarray(np.asarray(inputs["W_qkv"], dtype=np.float32))
    wout = np.ascontiguousarray(np.asarray(inputs["W_out"], dtype=np.float32))
    bout = np.ascontiguousarray(np.asarray(inputs["b_out"], dtype=np.float32))
    return [{"x": np.ascontiguousarray(x[i]), "pos": pos, "wqkv": wqkv,
             "wout": wout, "bout": bout} for i in range(B)]


def run(inputs, trace=False, **kwargs):
    """inputs: dict with full-shape arrays as in reference.setup_inputs()."""
    from concourse.bass_utils import run_bass_kernel_spmd
    nc = get_nc()
    res = run_bass_kernel_spmd(nc, make_in_maps(inputs),
                               core_ids=list(range(B)), trace=trace, **kwargs)
    out = np.stack([res.results[i]["y"] for i in range(B)], axis=0)
    return out, res


def kernel(**inputs):
    out, _ = run(inputs, trace=False)
    return out


# revision 19
# speedup vs baseline: 1.1833x; 1.0336x over previous
"""Rotary multi-head attention (b=8, n=1024, dim=768, heads=12, d_head=64)
on 8 Trainium2 NeuronCores, data-parallel over batch (1 batch row per core).

v8: coarse exp granularity + software-pipelined AV + paired PSUM parity.
  - exp at [128,1024] (96 ACTIVATEs): fine 512-granularity (v7) paid
    ~45% per-instruction Scalar overhead and ACT-paced the kernel.
  - Scores/exp EMITTED one step ahead of AV matmuls: the strict-FIFO PE
    queue never blocks on exp latency.
  - All non-stp PSUM users (qk acc, vp, pt, pp, sincos, op) allocate in
    ADJACENT PAIRS from the shared 2-slot "mm" tag, preserving rotation
    parity so score tiles never land on a slot gated by DVE rotary muls
    (v5's 3us LDWEIGHTS stalls).
  - Scalar runs only exp during attention (avsb copies on Vector, final
    divides on GpSimd, sin/cos copies on Vector).
  - Rotated q/k kept f32 through the swap; single end-stage bf16 round
    into qf; score matmuls read bf16 (FWL).
  - Normalize: batched [128,16] reciprocal, DRAM bounce, one [64,2048]
    broadcast read; den writes + rep read on one queue (FIFO ordering).
"""
import sys
import numpy as np

if '/opt/trn_rl_repo' not in sys.path:
    sys.path.insert(0, '/opt/trn_rl_repo')

B, N, DIM = 8, 1024, 768
HEADS, DHEAD = 12, 64
INNER = HEADS * DHEAD           # 768
SCALE = DHEAD ** -0.5           # 0.125
NCH = N // 128                  # 8 n-chunks
KCH = DIM // 128                # 6 contraction chunks
TCH = HEADS // 2                # 6 head pairs

_CACHE = {}


def _build():
    import concourse.mybir as mybir
    from concourse import bacc
    from concourse.tile import TileContext

    F32 = mybir.dt.float32
    F32R = mybir.dt.float32r
    BF16 = mybir.dt.bfloat16
    AF = mybir.ActivationFunctionType

    nc = bacc.Bacc("TRN2", target_bir_lowering=False, debug=False, num_devices=8)

    x_d = nc.dram_tensor("x", [N, DIM], F32, kind="ExternalInput")
    pos_d = nc.dram_tensor("pos", [N, DHEAD], F32, kind="ExternalInput")
    wqkv_d = nc.dram_tensor("wqkv", [DIM, 3 * INNER], F32R, kind="ExternalInput")
    wout_d = nc.dram_tensor("wout", [INNER, DIM], F32, kind="ExternalInput")
    bout_d = nc.dram_tensor("bout", [DIM], F32, kind="ExternalInput")
    y_d = nc.dram_tensor("y", [N, DIM], F32, kind="ExternalOutput")
    den_d = nc.dram_tensor("den_scr", [TCH, 4, 128, 4], F32)

    # ---- inline constants -------------------------------------------------
    ident_d = nc.inline_tensor(np.eye(128, dtype=np.float32), name="ident")
    # sin table with pair-swap sign baked in: odd rows get -sin.
    # posT rows 0:32 = sin values, 32:64 = cos values.
    RsinT = np.zeros((64, 128), np.float32)
    RcosT = np.zeros((64, 128), np.float32)
    for m in range(128):
        RsinT[(m % 64) // 2, m] = -1.0 if (m % 2 == 1) else 1.0
        RcosT[32 + (m % 64) // 2, m] = 1.0
    rsinT_d = nc.inline_tensor(RsinT, name="rsinT")
    rcosT_d = nc.inline_tensor(RcosT, name="rcosT")

    with TileContext(nc) as tc:
        with tc.tile_pool(name="wp", bufs=1) as wp, \
             tc.tile_pool(name="big", bufs=1) as big, \
             tc.tile_pool(name="tp", bufs=2) as tp, \
             tc.tile_pool(name="qpool", bufs=2) as qpool, \
             tc.tile_pool(name="epool", bufs=1) as epool, \
             tc.tile_pool(name="npool", bufs=2) as npool, \
             tc.tile_pool(name="misc", bufs=1) as misc, \
             tc.tile_pool(name="ps", bufs=2, space="PSUM") as ps, \
             tc.tile_pool(name="psv", bufs=4, space="PSUM") as psv:

            # ---- input DMAs.  sync queue: x, consts, Q+Klo weight half.
            # gpsimd queue: pos, rotary tables, Khi+V weight half, wout.
            ident_sb = misc.tile([128, 128], F32, name="ident_sb", tag="ident_sb")
            nc.sync.dma_start(ident_sb[:], ident_d.ap())
            b_row = misc.tile([1, DIM], F32, name="b_row", tag="b_row")
            nc.sync.dma_start(b_row[:], bout_d.ap().unsqueeze(0))
            x_sb = []
            for i in range(NCH):
                xs = tp.tile([128, DIM], F32, name=f"x_sb_{i}", tag="xsb", bufs=3)
                nc.sync.dma_start(xs[:], x_d[i * 128:(i + 1) * 128, :])
                x_sb.append(xs)

            p_all = misc.tile([128, NCH * DHEAD], F32, name="p_all", tag="p_all")
            for i in range(NCH):
                nc.gpsimd.dma_start(p_all[:, i * DHEAD:(i + 1) * DHEAD],
                                    pos_d[i * 128:(i + 1) * 128, :])

            b_bcast = misc.tile([128, DIM], F32, name="b_bcast", tag="b_bcast")
            nc.gpsimd.partition_broadcast(b_bcast[:], b_row[:])

            rs_sb = misc.tile([64, 128], F32R, name="rs_sb", tag="rs_sb")
            nc.gpsimd.dma_start(rs_sb[:], rsinT_d.ap())
            rc_sb = misc.tile([64, 128], F32R, name="rc_sb", tag="rc_sb")
            nc.gpsimd.dma_start(rc_sb[:], rcosT_d.ap())

            wqkv_r = [wp.tile([128, 3 * INNER], F32R, name=f"wqkv_r_{k}",
                              tag=f"wqkv_r_{k}") for k in range(KCH)]
            half_cols = 3 * INNER // 2      # 1152
            for k in range(KCH):
                nc.sync.dma_start(wqkv_r[k][:, 0:half_cols],
                                  wqkv_d[k * 128:(k + 1) * 128, 0:half_cols])
                nc.gpsimd.dma_start(wqkv_r[k][:, half_cols:],
                                    wqkv_d[k * 128:(k + 1) * 128, half_cols:])
            # wout cast to bf16 by the software DGE
            wout_b = [wp.tile([128, DIM], BF16, name=f"wout_b_{k}",
                              tag=f"wout_b_{k}") for k in range(KCH)]
            for k in range(KCH):
                nc.gpsimd.dma_start(wout_b[k][:], wout_d[k * 128:(k + 1) * 128, :])

            # ---- vaug memset (ones columns give softmax denominators)
            vaug_all = big.tile([128, NCH * HEADS * 65], BF16, name="vaug",
                                tag="vaug")
            vaug = [vaug_all[:, i * HEADS * 65:(i + 1) * HEADS * 65]
                    for i in range(NCH)]
            nc.gpsimd.memset(vaug_all[:], 1.0)

            # preload the gpsimd tensor-tensor library off the critical path
            dummy = misc.tile([1, 16], F32, name="dummy_tt", tag="dummy_tt")
            nc.gpsimd.memset(dummy[:], 1.0)
            nc.gpsimd.tensor_add(dummy[:], dummy[:], dummy[:])

            # ---- X^T via PE transposes; PSUM->SBUF copies alternate
            # Scalar/Vector.  pt emitted as an adjacent PAIR (parity).
            xt = [big.tile([128, N], F32R, name=f"xt{k}", tag=f"xt{k}")
                  for k in range(KCH)]

            def emit_transpose_chunk(i):
                pta = ps.tile([128, 512], F32, name=f"pta_{i}", tag="mm")
                ptb = ps.tile([128, 512], F32, name=f"ptb_{i}", tag="mm")
                for k in range(KCH):
                    dst = pta[:, k * 128:(k + 1) * 128] if k < 4 \
                        else ptb[:, (k - 4) * 128:(k - 3) * 128]
                    nc.tensor.transpose(dst,
                                        x_sb[i][:, k * 128:(k + 1) * 128],
                                        ident_sb[:])
                for k in range(KCH):
                    src = pta[:, k * 128:(k + 1) * 128] if k < 4 \
                        else ptb[:, (k - 4) * 128:(k - 3) * 128]
                    if (i * KCH + k) % 2 == 0:
                        nc.scalar.activation(xt[k][:, i * 128:(i + 1) * 128],
                                             src, AF.Copy)
                    else:
                        nc.vector.tensor_copy(xt[k][:, i * 128:(i + 1) * 128],
                                              src)

            for i in range(4):
                emit_transpose_chunk(i)

            # ---- pos -> posT -> sin128 (signed) / cos128
            posT = misc.tile([64, N], F32R, name="posT", tag="posT")
            sin128 = misc.tile([128, N], F32, name="sin128", tag="sin128")
            cos128 = misc.tile([128, N], F32, name="cos128", tag="cos128")

            def emit_sincos():
                pps = []
                for half in range(2):
                    pp = ps.tile([64, 512], F32, name=f"pp{half}", tag="mm")
                    pps.append(pp)
                    for i2 in range(4):
                        i = half * 4 + i2
                        nc.tensor.transpose(pp[:, i2 * 128:(i2 + 1) * 128],
                                            p_all[:, i * 64:(i + 1) * 64],
                                            ident_sb[:])
                for half in range(2):
                    sl = slice(half * 512, (half + 1) * 512)
                    nc.scalar.activation(posT[:, sl], pps[half][:], AF.Copy)
                for half in range(2):
                    sl = slice(half * 512, (half + 1) * 512)
                    ps_s = ps.tile([128, 512], F32, name=f"ps_s{half}", tag="mm")
                    nc.tensor.matmul(ps_s[:], rs_sb[:], posT[:, sl],
                                     start=True, stop=True)
                    ps_c = ps.tile([128, 512], F32, name=f"ps_c{half}", tag="mm")
                    nc.tensor.matmul(ps_c[:], rc_sb[:], posT[:, sl],
                                     start=True, stop=True)
                    nc.vector.tensor_copy(sin128[:, sl], ps_s[:])
                    nc.vector.tensor_copy(cos128[:, sl], ps_c[:])

            # ---- helpers ----------------------------------------------------
            qk_tiles = {}
            qk_parts = {}

            def emit_qk_half(t, which, half):
                """One n-half of rotated q/k production for pair t."""
                cbase = t * 128 if which == "qA" else INNER + t * 128
                sl = slice(half * 512, (half + 1) * 512)
                if (t, which) not in qk_parts:
                    qs = qpool.tile([128, N], F32, name=f"qs_{which}_{t}",
                                    tag="qs", bufs=2)
                    qsw = qpool.tile([128, N], F32, name=f"qsw_{which}_{t}",
                                     tag="qsw", bufs=2)
                    qx = qpool.tile([128, N], F32, name=f"qx_{which}_{t}",
                                    tag=f"qx_{which}", bufs=1)
                    qf = qpool.tile([128, N], BF16, name=f"{which}_{t}",
                                    tag=which, bufs=2)
                    qk_parts[(t, which)] = (qs, qsw, qx, qf)
                    qk_tiles[(t, which)] = qf
                qs, qsw, qx, qf = qk_parts[(t, which)]
                acc = ps.tile([128, 512], F32, name=f"qk_{which}_{t}_{half}",
                              tag="mm")
                for k in range(KCH):
                    nc.tensor.matmul(acc[:], wqkv_r[k][:, cbase:cbase + 128],
                                     xt[k][:, sl],
                                     start=(k == 0), stop=(k == KCH - 1))
                nc.vector.tensor_mul(qs[:, sl], acc[:], sin128[:, sl])
                nc.vector.tensor_mul(qx[:, sl], acc[:], cos128[:, sl])
                if half == 1:
                    # pair-swap partitions on the full tile (sliced
                    # partition-split DMA APs mis-lower); one per queue
                    qs_v = qs[:].rearrange("(a two) n -> two a n", two=2)
                    qsw_v = qsw[:].rearrange("(a two) n -> two a n", two=2)
                    nc.sync.dma_start(qsw_v[0], qs_v[1])
                    nc.gpsimd.dma_start(qsw_v[1], qs_v[0])
                    for h2 in range(2):
                        s2 = slice(h2 * 512, (h2 + 1) * 512)
                        # f32 + f32 -> bf16 (single end-stage rounding)
                        nc.gpsimd.tensor_add(qf[:, s2], qx[:, s2], qsw[:, s2])

            def emit_qk_prod(t, which):
                """Both halves back-to-back: 2 adjacent 'mm' allocations."""
                emit_qk_half(t, which, 0)
                emit_qk_half(t, which, 1)

            def emit_v(jc):
                """V for n-chunk jc into ones-augmented vaug[jc] (bf16).
                Two adjacent 'mm' allocations (parity-preserving)."""
                for half in range(2):
                    vp = ps.tile([128, 512], F32, name=f"vp_{jc}_{half}",
                                 tag="mm")
                    for k in range(KCH):
                        nc.tensor.matmul(
                            vp[:, 0:384],
                            xt[k][:, jc * 128:(jc + 1) * 128],
                            wqkv_r[k][:, 2 * INNER + half * 384:
                                      2 * INNER + (half + 1) * 384],
                            start=(k == 0), stop=(k == KCH - 1))
                    dst = vaug[jc][:, half * 390:half * 390 + 390] \
                        .rearrange("p (h d) -> p h d", d=65)[:, :, 0:64]
                    src = vp[:, 0:384].rearrange("p (h d) -> p h d", d=64)
                    nc.vector.tensor_copy(dst, src)

            ao = [big.tile([128, N], BF16, name=f"ao{t}", tag=f"ao{t}")
                  for t in range(TCH)]

            def emit_normalize(t, avsb):
                """Divide AV by denominators (row 64) into bf16 ao[t]."""
                order = ((0, 0), (1, 0), (0, 1), (1, 1))
                dsq = npool.tile([128, 16], F32, name=f"dsq_{t}", tag="dsq")
                for c, (h, nh) in enumerate(order):
                    eng = nc.sync if c % 2 == 0 else nc.gpsimd
                    eng.dma_start(dsq[:, 4 * c:4 * (c + 1)],
                                  avsb[h][nh][64:65, :])
                nc.vector.reciprocal(dsq[:], dsq[:])
                # den writes + rep read on the SAME queue (gpsimd) so SWDGE
                # FIFO order guarantees write-before-read in DRAM
                for c in range(4):
                    nc.gpsimd.dma_start(den_d.ap()[t][c],
                                        dsq[:, 4 * c:4 * (c + 1)])
                rep = npool.tile([64, 4 * 512], F32, name=f"rep_{t}", tag="rep",
                                 bufs=1)
                nc.gpsimd.dma_start(
                    rep[:],
                    den_d.ap()[t].rearrange("c p q -> (c p q)")
                    .partition_broadcast(64))
                # divides on GpSimd (all-SBUF): keeps the Vector FIFO clear
                for c, (h, nh) in enumerate(order):
                    nc.gpsimd.tensor_mul(
                        ao[t][h * 64:(h + 1) * 64, nh * 512:(nh + 1) * 512],
                        avsb[h][nh][0:64, :],
                        rep[:, c * 512:(c + 1) * 512])

            def emit_attention(t, interleave):
                """Attention for pair t.  Scores/exp emitted one step ahead
                of the AV matmuls (software pipeline over jc).
                interleave: dict step -> [callables]."""
                qa = qk_tiles[(t, "qA")]
                qb = qk_tiles[(t, "qB")]
                avsb = [[None, None], [None, None]]
                av = [[None, None], [None, None]]
                es = {}
                for step in range(NCH + 2):
                    if step < NCH:
                        jc = step
                        jsl = slice(jc * 128, (jc + 1) * 128)
                        stp0 = ps.tile([128, N], F32, name=f"stp0_{t}_{jc}",
                                       tag="mm")
                        stp1 = ps.tile([128, N], F32, name=f"stp1_{t}_{jc}",
                                       tag="mm")
                        for hf in range(2):
                            sl = slice(hf * 512, (hf + 1) * 512)
                            nc.tensor.matmul(stp0[:, sl], qb[0:64, jsl],
                                             qa[0:64, sl],
                                             start=True, stop=True,
                                             tile_position=(0, 0))
                            nc.tensor.matmul(stp1[:, sl], qb[64:128, jsl],
                                             qa[64:128, sl],
                                             start=True, stop=True,
                                             tile_position=(64, 0))
                        e0 = epool.tile([128, N], BF16, name=f"e0_{t}_{jc}",
                                        tag="e", bufs=5)
                        nc.scalar.activation(e0[:], stp0[:], AF.Exp, scale=SCALE)
                        e1 = epool.tile([128, N], BF16, name=f"e1_{t}_{jc}",
                                        tag="e", bufs=5)
                        nc.scalar.activation(e1[:], stp1[:], AF.Exp, scale=SCALE)
                        es[jc] = (e0, e1)
                        if jc == 0:
                            for h in range(2):
                                for nh in range(2):
                                    av[h][nh] = psv.tile(
                                        [65, 512], F32, name=f"av_{t}_{h}_{nh}",
                                        tag="avs")
                    if step > 1:
                        jc = step - 2
                        e0, e1 = es.pop(jc)
                        for h, e in ((0, e0), (1, e1)):
                            vsl = vaug[jc][:, (2 * t + h) * 65:
                                           (2 * t + h + 1) * 65]
                            for nh in range(2):
                                nc.tensor.matmul(av[h][nh][:], vsl,
                                                 e[:, nh * 512:(nh + 1) * 512],
                                                 start=(jc == 0),
                                                 stop=(jc == NCH - 1))
                    for fn in interleave.get(step, ()):
                        fn()
                for h in range(2):
                    for nh in range(2):
                        sb = npool.tile([65, 512], F32, name=f"avsb_{t}_{h}_{nh}",
                                        tag="avsb", bufs=4)
                        nc.vector.tensor_copy(sb[:], av[h][nh][:])
                        avsb[h][nh] = sb
                emit_normalize(t, avsb)

            # ---- schedule ---------------------------------------------------
            emit_transpose_chunk(4)
            emit_transpose_chunk(5)
            emit_transpose_chunk(6)
            emit_transpose_chunk(7)
            emit_sincos()
            emit_qk_prod(0, "qA")
            emit_qk_prod(0, "qB")
            emit_v(0)
            emit_qk_prod(1, "qA")
            emit_qk_prod(1, "qB")

            for t in range(TCH):
                inter = {}
                if t == 0:
                    inter = {jc: [lambda jc=jc: emit_v(jc + 1)]
                             for jc in range(7)}
                elif t < TCH - 1:
                    tn = t + 1
                    inter = {2: [lambda tn=tn: emit_qk_prod(tn, "qA")],
                             5: [lambda tn=tn: emit_qk_prod(tn, "qB")]}
                emit_attention(t, inter)

            # ---- output projection + bias
            for i in range(NCH):
                op_lo = ps.tile([128, 512], F32, name=f"oplo_{i}", tag="mm")
                op_hi = ps.tile([128, 512], F32, name=f"ophi_{i}", tag="mm")
                for k in range(KCH):
                    lhs = ao[k][:, i * 128:(i + 1) * 128]
                    nc.tensor.matmul(op_lo[:], lhs, wout_b[k][:, 0:512],
                                     start=(k == 0), stop=(k == KCH - 1))
                    nc.tensor.matmul(op_hi[:, 0:256], lhs, wout_b[k][:, 512:768],
                                     start=(k == 0), stop=(k == KCH - 1))
                y_sb = tp.tile([128, DIM], F32, name=f"y_sb_{i}", tag="ysb", bufs=2)
                nc.vector.tensor_add(y_sb[:, 0:512], op_lo[:], b_bcast[:, 0:512])
                nc.vector.tensor_add(y_sb[:, 512:768], op_hi[:, 0:256],
                                     b_bcast[:, 512:768])
                nc.sync.dma_start(y_d[i * 128:(i + 1) * 128, :], y_sb[:])

    nc.compile()
    return nc


def get_nc():
    if 'nc' not in _CACHE:
        _CACHE['nc'] = _build()
    return _CACHE['nc']


def make_in_maps(inputs):
    x = np.ascontiguousarray(np.asarray(inputs["x"], dtype=np.float32))
    pos = np.ascontiguousarray(
        np.asarray(inputs["pos_emb"], dtype=np.float32).reshape(N, DHEAD))
    wqkv = np.ascontiguousarray(np.asarray(inputs["W_qkv"], dtype=np.float32))
    wout = np.ascontiguousarray(np.asarray(inputs["W_out"], dtype=np.float32))
    bout = np.ascontiguousarray(np.asarray(inputs["b_out"], dtype=np.float32))
    return [{"x": np.ascontiguousarray(x[i]), "pos": pos, "wqkv": wqkv,
             "wout": wout, "bout": bout} for i in range(B)]


def run(inputs, trace=False, **kwargs):
    """inputs: dict with full-shape arrays as in reference.setup_inputs()."""
    from concourse.bass_utils import run_bass_kernel_spmd
    nc = get_nc()
    res = run_bass_kernel_spmd(nc, make_in_maps(inputs),
                               core_ids=list(range(B)), trace=trace, **kwargs)
    out = np.stack([res.results[i]["y"] for i in range(B)], axis=0)
    return out, res


def kernel(**inputs):
    out, _ = run(inputs, trace=False)
    return out


# revision 24
# speedup vs baseline: 1.2121x; 1.0243x over previous
"""Rotary multi-head attention (b=8, n=1024, dim=768, heads=12, d_head=64)
on 8 Trainium2 NeuronCores, data-parallel over batch (1 batch row per core).

v8: coarse exp granularity + software-pipelined AV + paired PSUM parity.
  - exp at [128,1024] (96 ACTIVATEs): fine 512-granularity (v7) paid
    ~45% per-instruction Scalar overhead and ACT-paced the kernel.
  - Scores/exp EMITTED one step ahead of AV matmuls: the strict-FIFO PE
    queue never blocks on exp latency.
  - All non-stp PSUM users (qk acc, vp, pt, pp, sincos, op) allocate in
    ADJACENT PAIRS from the shared 2-slot "mm" tag, preserving rotation
    parity so score tiles never land on a slot gated by DVE rotary muls
    (v5's 3us LDWEIGHTS stalls).
  - Scalar runs only exp during attention (avsb copies on Vector, final
    divides on GpSimd, sin/cos copies on Vector).
  - Rotated q/k kept f32 through the swap; single end-stage bf16 round
    into qf; score matmuls read bf16 (FWL).
  - Normalize: batched [128,16] reciprocal, DRAM bounce, one [64,2048]
    broadcast read; den writes + rep read on one queue (FIFO ordering).
"""
import sys
import numpy as np

if '/opt/trn_rl_repo' not in sys.path:
    sys.path.insert(0, '/opt/trn_rl_repo')

B, N, DIM = 8, 1024, 768
HEADS, DHEAD = 12, 64
INNER = HEADS * DHEAD           # 768
SCALE = DHEAD ** -0.5           # 0.125
NCH = N // 128                  # 8 n-chunks
KCH = DIM // 128                # 6 contraction chunks
TCH = HEADS // 2                # 6 head pairs

_CACHE = {}


def _build():
    import concourse.mybir as mybir
    from concourse import bacc
    from concourse.tile import TileContext

    F32 = mybir.dt.float32
    F32R = mybir.dt.float32r
    BF16 = mybir.dt.bfloat16
    AF = mybir.ActivationFunctionType

    nc = bacc.Bacc("TRN2", target_bir_lowering=False, debug=False, num_devices=8)

    x_d = nc.dram_tensor("x", [N, DIM], F32, kind="ExternalInput")
    pos_d = nc.dram_tensor("pos", [N, DHEAD], F32, kind="ExternalInput")
    wqkv_d = nc.dram_tensor("wqkv", [DIM, 3 * INNER], F32R, kind="ExternalInput")
    wout_d = nc.dram_tensor("wout", [INNER, DIM], F32, kind="ExternalInput")
    bout_d = nc.dram_tensor("bout", [DIM], F32, kind="ExternalInput")
    y_d = nc.dram_tensor("y", [N, DIM], F32, kind="ExternalOutput")
    den_d = nc.dram_tensor("den_scr", [TCH, 4, 128, 4], F32)

    # ---- inline constants -------------------------------------------------
    ident_d = nc.inline_tensor(np.eye(128, dtype=np.float32), name="ident")
    # sin table with pair-swap sign baked in: odd rows get -sin.
    # posT rows 0:32 = sin values, 32:64 = cos values.
    RsinT = np.zeros((64, 128), np.float32)
    RcosT = np.zeros((64, 128), np.float32)
    for m in range(128):
        RsinT[(m % 64) // 2, m] = -1.0 if (m % 2 == 1) else 1.0
        RcosT[32 + (m % 64) // 2, m] = 1.0
    rsinT_d = nc.inline_tensor(RsinT, name="rsinT")
    rcosT_d = nc.inline_tensor(RcosT, name="rcosT")

    with TileContext(nc) as tc:
        with tc.tile_pool(name="wp", bufs=1) as wp, \
             tc.tile_pool(name="big", bufs=1) as big, \
             tc.tile_pool(name="tp", bufs=2) as tp, \
             tc.tile_pool(name="qpool", bufs=2) as qpool, \
             tc.tile_pool(name="epool", bufs=1) as epool, \
             tc.tile_pool(name="npool", bufs=2) as npool, \
             tc.tile_pool(name="misc", bufs=1) as misc, \
             tc.tile_pool(name="ps", bufs=2, space="PSUM") as ps, \
             tc.tile_pool(name="psv", bufs=4, space="PSUM") as psv:

            # ---- input DMAs.  sync queue: x, consts, Q+Klo weight half.
            # gpsimd queue: pos, rotary tables, Khi+V weight half, wout.
            ident_sb = misc.tile([128, 128], F32, name="ident_sb", tag="ident_sb")
            nc.sync.dma_start(ident_sb[:], ident_d.ap())
            b_row = misc.tile([1, DIM], F32, name="b_row", tag="b_row")
            nc.sync.dma_start(b_row[:], bout_d.ap().unsqueeze(0))
            x_sb = []
            for i in range(NCH):
                xs = tp.tile([128, DIM], F32, name=f"x_sb_{i}", tag="xsb", bufs=3)
                nc.sync.dma_start(xs[:], x_d[i * 128:(i + 1) * 128, :])
                x_sb.append(xs)

            # pos + rotary tables on the (preamble-idle) scalar queue
            p_all = misc.tile([128, NCH * DHEAD], F32, name="p_all", tag="p_all")
            for i in range(NCH):
                nc.scalar.dma_start(p_all[:, i * DHEAD:(i + 1) * DHEAD],
                                    pos_d[i * 128:(i + 1) * 128, :])

            rs_sb = misc.tile([64, 128], F32R, name="rs_sb", tag="rs_sb")
            nc.gpsimd.dma_start(rs_sb[:], rsinT_d.ap())
            rc_sb = misc.tile([64, 128], F32R, name="rc_sb", tag="rc_sb")
            nc.gpsimd.dma_start(rc_sb[:], rcosT_d.ap())

            # gpsimd ENGINE work first (no DMA deps): vaug memset + library
            # preload + bias broadcast -- runs during the x DMA.
            vaug_all = big.tile([128, NCH * HEADS * 65], BF16, name="vaug",
                                tag="vaug")
            vaug = [vaug_all[:, i * HEADS * 65:(i + 1) * HEADS * 65]
                    for i in range(NCH)]
            nc.gpsimd.memset(vaug_all[:], 1.0)
            dummy = misc.tile([1, 16], F32, name="dummy_tt", tag="dummy_tt")
            nc.gpsimd.memset(dummy[:], 1.0)
            nc.gpsimd.tensor_add(dummy[:], dummy[:], dummy[:])
            b_bcast = misc.tile([128, DIM], F32, name="b_bcast", tag="b_bcast")
            nc.gpsimd.partition_broadcast(b_bcast[:], b_row[:])

            wqkv_r = [wp.tile([128, 3 * INNER], F32R, name=f"wqkv_r_{k}",
                              tag=f"wqkv_r_{k}") for k in range(KCH)]
            half_cols = 3 * INNER // 2      # 1152
            for k in range(KCH):
                nc.sync.dma_start(wqkv_r[k][:, 0:half_cols],
                                  wqkv_d[k * 128:(k + 1) * 128, 0:half_cols])
                nc.gpsimd.dma_start(wqkv_r[k][:, half_cols:],
                                    wqkv_d[k * 128:(k + 1) * 128, half_cols:])
            # wout (cast to bf16: gpsimd-only) -- EMITTED LATE in the t loop
            # so it never delays preamble-critical gpsimd work
            wout_b = [wp.tile([128, DIM], BF16, name=f"wout_b_{k}",
                              tag=f"wout_b_{k}") for k in range(KCH)]

            def emit_wout_loads():
                for k in range(KCH):
                    nc.gpsimd.dma_start(wout_b[k][:],
                                        wout_d[k * 128:(k + 1) * 128, :])

            # ---- X^T via PE transposes; PSUM->SBUF copies alternate
            # Scalar/Vector.  pt emitted as an adjacent PAIR (parity).
            xt = [big.tile([128, N], F32R, name=f"xt{k}", tag=f"xt{k}")
                  for k in range(KCH)]

            def emit_transpose_chunk(i):
                pta = ps.tile([128, 512], F32, name=f"pta_{i}", tag="mm")
                ptb = ps.tile([128, 512], F32, name=f"ptb_{i}", tag="mm")
                for k in range(KCH):
                    dst = pta[:, k * 128:(k + 1) * 128] if k < 4 \
                        else ptb[:, (k - 4) * 128:(k - 3) * 128]
                    nc.tensor.transpose(dst,
                                        x_sb[i][:, k * 128:(k + 1) * 128],
                                        ident_sb[:])
                for k in range(KCH):
                    src = pta[:, k * 128:(k + 1) * 128] if k < 4 \
                        else ptb[:, (k - 4) * 128:(k - 3) * 128]
                    if (i * KCH + k) % 2 == 0:
                        nc.scalar.activation(xt[k][:, i * 128:(i + 1) * 128],
                                             src, AF.Copy)
                    else:
                        nc.vector.tensor_copy(xt[k][:, i * 128:(i + 1) * 128],
                                              src)

            for i in range(4):
                emit_transpose_chunk(i)

            # ---- pos -> posT -> sin128 (signed) / cos128
            posT = misc.tile([64, N], F32R, name="posT", tag="posT")
            sin128 = misc.tile([128, N], F32, name="sin128", tag="sin128")
            cos128 = misc.tile([128, N], F32, name="cos128", tag="cos128")

            def emit_sincos():
                pps = []
                for half in range(2):
                    pp = ps.tile([64, 512], F32, name=f"pp{half}", tag="mm")
                    pps.append(pp)
                    for i2 in range(4):
                        i = half * 4 + i2
                        nc.tensor.transpose(pp[:, i2 * 128:(i2 + 1) * 128],
                                            p_all[:, i * 64:(i + 1) * 64],
                                            ident_sb[:])
                for half in range(2):
                    sl = slice(half * 512, (half + 1) * 512)
                    nc.scalar.activation(posT[:, sl], pps[half][:], AF.Copy)
                for half in range(2):
                    sl = slice(half * 512, (half + 1) * 512)
                    ps_s = ps.tile([128, 512], F32, name=f"ps_s{half}", tag="mm")
                    nc.tensor.matmul(ps_s[:], rs_sb[:], posT[:, sl],
                                     start=True, stop=True)
                    ps_c = ps.tile([128, 512], F32, name=f"ps_c{half}", tag="mm")
                    nc.tensor.matmul(ps_c[:], rc_sb[:], posT[:, sl],
                                     start=True, stop=True)
                    nc.vector.tensor_copy(sin128[:, sl], ps_s[:])
                    nc.vector.tensor_copy(cos128[:, sl], ps_c[:])

            # ---- helpers ----------------------------------------------------
            qk_tiles = {}
            qk_parts = {}

            def emit_qk_half(t, which, half):
                """One n-half of rotated q/k production for pair t."""
                cbase = t * 128 if which == "qA" else INNER + t * 128
                sl = slice(half * 512, (half + 1) * 512)
                if (t, which) not in qk_parts:
                    qs = qpool.tile([128, N], F32, name=f"qs_{which}_{t}",
                                    tag="qs", bufs=2)
                    qsw = qpool.tile([128, N], F32, name=f"qsw_{which}_{t}",
                                     tag="qsw", bufs=2)
                    qx = qpool.tile([128, N], F32, name=f"qx_{which}_{t}",
                                    tag=f"qx_{which}", bufs=1)
                    qf = qpool.tile([128, N], BF16, name=f"{which}_{t}",
                                    tag=which, bufs=2)
                    qk_parts[(t, which)] = (qs, qsw, qx, qf)
                    qk_tiles[(t, which)] = qf
                qs, qsw, qx, qf = qk_parts[(t, which)]
                acc = ps.tile([128, 512], F32, name=f"qk_{which}_{t}_{half}",
                              tag="mm")
                for k in range(KCH):
                    nc.tensor.matmul(acc[:], wqkv_r[k][:, cbase:cbase + 128],
                                     xt[k][:, sl],
                                     start=(k == 0), stop=(k == KCH - 1))
                nc.vector.tensor_mul(qs[:, sl], acc[:], sin128[:, sl])
                nc.vector.tensor_mul(qx[:, sl], acc[:], cos128[:, sl])
                if half == 1:
                    # pair-swap partitions on the full tile (sliced
                    # partition-split DMA APs mis-lower); one per queue
                    qs_v = qs[:].rearrange("(a two) n -> two a n", two=2)
                    qsw_v = qsw[:].rearrange("(a two) n -> two a n", two=2)
                    nc.sync.dma_start(qsw_v[0], qs_v[1])
                    nc.gpsimd.dma_start(qsw_v[1], qs_v[0])
                    for h2 in range(2):
                        s2 = slice(h2 * 512, (h2 + 1) * 512)
                        # f32 + f32 -> bf16 (single end-stage rounding)
                        nc.gpsimd.tensor_add(qf[:, s2], qx[:, s2], qsw[:, s2])

            def emit_qk_prod(t, which):
                """Both halves back-to-back: 2 adjacent 'mm' allocations."""
                emit_qk_half(t, which, 0)
                emit_qk_half(t, which, 1)

            def emit_v(jc):
                """V for n-chunk jc into ones-augmented vaug[jc] (bf16).
                Two adjacent 'mm' allocations (parity-preserving)."""
                for half in range(2):
                    vp = ps.tile([128, 512], F32, name=f"vp_{jc}_{half}",
                                 tag="mm")
                    for k in range(KCH):
                        nc.tensor.matmul(
                            vp[:, 0:384],
                            xt[k][:, jc * 128:(jc + 1) * 128],
                            wqkv_r[k][:, 2 * INNER + half * 384:
                                      2 * INNER + (half + 1) * 384],
                            start=(k == 0), stop=(k == KCH - 1))
                    dst = vaug[jc][:, half * 390:half * 390 + 390] \
                        .rearrange("p (h d) -> p h d", d=65)[:, :, 0:64]
                    src = vp[:, 0:384].rearrange("p (h d) -> p h d", d=64)
                    nc.vector.tensor_copy(dst, src)

            ao = [big.tile([128, N], BF16, name=f"ao{t}", tag=f"ao{t}")
                  for t in range(TCH)]

            NORM_ORDER = ((0, 0), (1, 0), (0, 1), (1, 1))

            def emit_normalize_phase1(t, avsb):
                """Denominator prep: gather, batched reciprocal, DRAM bounce,
                ONE [64,2048] broadcast read.  Emitted at pair end; latency
                is hidden because the divides run a pair later (phase2)."""
                dsq = npool.tile([128, 16], F32, name=f"dsq_{t}", tag="dsq")
                for c, (h, nh) in enumerate(NORM_ORDER):
                    eng = nc.sync if c % 2 == 0 else nc.gpsimd
                    eng.dma_start(dsq[:, 4 * c:4 * (c + 1)],
                                  avsb[h][nh][64:65, :])
                nc.vector.reciprocal(dsq[:], dsq[:])
                # den writes + rep read on the SAME queue (gpsimd) so SWDGE
                # FIFO order guarantees write-before-read in DRAM
                for c in range(4):
                    nc.gpsimd.dma_start(den_d.ap()[t][c],
                                        dsq[:, 4 * c:4 * (c + 1)])
                rep = npool.tile([64, 4 * 512], F32, name=f"rep_{t}", tag="rep",
                                 bufs=1)
                nc.gpsimd.dma_start(
                    rep[:],
                    den_d.ap()[t].rearrange("c p q -> (c p q)")
                    .partition_broadcast(64))
                return rep

            def emit_normalize_phase2(t, avsb, rep):
                """The 4 divides (GpSimd, all-SBUF) -- interleaved into the
                NEXT pair so the rep-DMA latency never blocks a queue head."""
                for c, (h, nh) in enumerate(NORM_ORDER):
                    nc.gpsimd.tensor_mul(
                        ao[t][h * 64:(h + 1) * 64, nh * 512:(nh + 1) * 512],
                        avsb[h][nh][0:64, :],
                        rep[:, c * 512:(c + 1) * 512])

            def emit_attention(t, interleave):
                """Attention for pair t.  Scores/exp emitted one step ahead
                of the AV matmuls (software pipeline over jc).
                interleave: dict step -> [callables]."""
                qa = qk_tiles[(t, "qA")]
                qb = qk_tiles[(t, "qB")]
                avsb = [[None, None], [None, None]]
                av = [[None, None], [None, None]]
                es = {}
                for step in range(NCH + 2):
                    if step < NCH:
                        jc = step
                        jsl = slice(jc * 128, (jc + 1) * 128)
                        stp0 = ps.tile([128, N], F32, name=f"stp0_{t}_{jc}",
                                       tag="mm")
                        stp1 = ps.tile([128, N], F32, name=f"stp1_{t}_{jc}",
                                       tag="mm")
                        for hf in range(2):
                            sl = slice(hf * 512, (hf + 1) * 512)
                            nc.tensor.matmul(stp0[:, sl], qb[0:64, jsl],
                                             qa[0:64, sl],
                                             start=True, stop=True,
                                             tile_position=(0, 0))
                            nc.tensor.matmul(stp1[:, sl], qb[64:128, jsl],
                                             qa[64:128, sl],
                                             start=True, stop=True,
                                             tile_position=(64, 0))
                        e0 = epool.tile([128, N], BF16, name=f"e0_{t}_{jc}",
                                        tag="e", bufs=5)
                        nc.scalar.activation(e0[:], stp0[:], AF.Exp, scale=SCALE)
                        e1 = epool.tile([128, N], BF16, name=f"e1_{t}_{jc}",
                                        tag="e", bufs=5)
                        nc.scalar.activation(e1[:], stp1[:], AF.Exp, scale=SCALE)
                        es[jc] = (e0, e1)
                        if jc == 0:
                            for h in range(2):
                                for nh in range(2):
                                    av[h][nh] = psv.tile(
                                        [65, 512], F32, name=f"av_{t}_{h}_{nh}",
                                        tag="avs")
                    if step > 1:
                        jc = step - 2
                        e0, e1 = es.pop(jc)
                        for h, e in ((0, e0), (1, e1)):
                            vsl = vaug[jc][:, (2 * t + h) * 65:
                                           (2 * t + h + 1) * 65]
                            for nh in range(2):
                                nc.tensor.matmul(av[h][nh][:], vsl,
                                                 e[:, nh * 512:(nh + 1) * 512],
                                                 start=(jc == 0),
                                                 stop=(jc == NCH - 1))
                    for fn in interleave.get(step, ()):
                        fn()
                for h in range(2):
                    for nh in range(2):
                        sb = npool.tile([65, 512], F32, name=f"avsb_{t}_{h}_{nh}",
                                        tag="avsb", bufs=4)
                        nc.vector.tensor_copy(sb[:], av[h][nh][:])
                        avsb[h][nh] = sb
                rep = emit_normalize_phase1(t, avsb)
                return avsb, rep

            # ---- schedule ---------------------------------------------------
            emit_transpose_chunk(4)
            emit_transpose_chunk(5)
            emit_transpose_chunk(6)
            emit_transpose_chunk(7)
            emit_sincos()
            emit_qk_prod(0, "qA")
            emit_qk_prod(0, "qB")
            emit_v(0)
            emit_qk_prod(1, "qA")
            emit_qk_prod(1, "qB")

            prev_norm = None        # (t-1, avsb, rep) pending divides
            for t in range(TCH):
                inter = {}
                if t == 0:
                    inter = {jc: [lambda jc=jc: emit_v(jc + 1)]
                             for jc in range(7)}
                elif t < TCH - 1:
                    tn = t + 1
                    inter = {2: [lambda tn=tn: emit_qk_prod(tn, "qA")],
                             5: [lambda tn=tn: emit_qk_prod(tn, "qB")]}
                if prev_norm is not None:
                    tp_, avsb_, rep_ = prev_norm
                    inter.setdefault(3, []).append(
                        lambda tp_=tp_, avsb_=avsb_, rep_=rep_:
                        emit_normalize_phase2(tp_, avsb_, rep_))
                if t == 3:
                    inter.setdefault(6, []).append(emit_wout_loads)
                avsb_t, rep_t = emit_attention(t, inter)
                prev_norm = (t, avsb_t, rep_t)

            # ---- output projection + bias.
            # Chunks 0-1 pre-accumulate k=0..4 (ao[5]-independent) so the PE
            # stays busy/warm while pair 5's normalize chain completes.
            # op tiles use the freed "avs" PSUM slots (4 x 1 bank).
            op_tiles = {}
            for i in range(2):
                op_lo = psv.tile([128, 512], F32, name=f"oplo_{i}", tag="avs")
                op_hi = psv.tile([128, 512], F32, name=f"ophi_{i}", tag="avs")
                op_tiles[i] = (op_lo, op_hi)
                for k in range(KCH - 1):
                    lhs = ao[k][:, i * 128:(i + 1) * 128]
                    nc.tensor.matmul(op_lo[:], lhs, wout_b[k][:, 0:512],
                                     start=(k == 0), stop=False)
                    nc.tensor.matmul(op_hi[:, 0:256], lhs, wout_b[k][:, 512:768],
                                     start=(k == 0), stop=False)
            # last pair's divides (rep already in flight)
            t5, avsb5, rep5 = prev_norm
            emit_normalize_phase2(t5, avsb5, rep5)

            def finish_chunk(i, op_lo, op_hi, k0):
                for k in range(k0, KCH):
                    lhs = ao[k][:, i * 128:(i + 1) * 128]
                    nc.tensor.matmul(op_lo[:], lhs, wout_b[k][:, 0:512],
                                     start=(k == 0), stop=(k == KCH - 1))
                    nc.tensor.matmul(op_hi[:, 0:256], lhs, wout_b[k][:, 512:768],
                                     start=(k == 0), stop=(k == KCH - 1))
                y_sb = tp.tile([128, DIM], F32, name=f"y_sb_{i}", tag="ysb",
                               bufs=2)
                nc.vector.tensor_add(y_sb[:, 0:512], op_lo[:], b_bcast[:, 0:512])
                nc.vector.tensor_add(y_sb[:, 512:768], op_hi[:, 0:256],
                                     b_bcast[:, 512:768])
                nc.sync.dma_start(y_d[i * 128:(i + 1) * 128, :], y_sb[:])

            for i in range(2):
                op_lo, op_hi = op_tiles[i]
                finish_chunk(i, op_lo, op_hi, KCH - 1)
            for i in range(2, NCH):
                op_lo = psv.tile([128, 512], F32, name=f"oplo_{i}", tag="avs")
                op_hi = psv.tile([128, 512], F32, name=f"ophi_{i}", tag="avs")
                finish_chunk(i, op_lo, op_hi, 0)

    nc.compile()
    return nc


def get_nc():
    if 'nc' not in _CACHE:
        _CACHE['nc'] = _build()
    return _CACHE['nc']


def make_in_maps(inputs):
    x = np.ascontiguousarray(np.asarray(inputs["x"], dtype=np.float32))
    pos = np.ascontiguousarray(
        np.asarray(inputs["pos_emb"], dtype=np.float32).reshape(N, DHEAD))
    wqkv = np.ascontiguousarray(np.asarray(inputs["W_qkv"], dtype=np.float32))
    wout = np.ascontiguousarray(np.asarray(inputs["W_out"], dtype=np.float32))
    bout = np.ascontiguousarray(np.asarray(inputs["b_out"], dtype=np.float32))
    return [{"x": np.ascontiguousarray(x[i]), "pos": pos, "wqkv": wqkv,
             "wout": wout, "bout": bout} for i in range(B)]


def run(inputs, trace=False, **kwargs):
    """inputs: dict with full-shape arrays as in reference.setup_inputs()."""
    from concourse.bass_utils import run_bass_kernel_spmd
    nc = get_nc()
    res = run_bass_kernel_spmd(nc, make_in_maps(inputs),
                               core_ids=list(range(B)), trace=trace, **kwargs)
    out = np.stack([res.results[i]["y"] for i in range(B)], axis=0)
    return out, res


def kernel(**inputs):
    out, _ = run(inputs, trace=False)
    return out
